# revision 100
# baseline (speedup 1.0000x reference)
"""Two-layer GAT (PyG GATConv-style) on 8 Trainium2 NeuronCores via Bass/Tile.

Edges-on-partitions design (v3, pair-window pipeline):
  - Nodes are degree-stratified into 98 strata of 512; each stratum contributes
    64 nodes to every core (snake order), giving core-major table rows
    row = core*6272 + window*64 + slot.  Window = 64 destination nodes.
  - A bf16 feature table holds rows [h(256) | alpha_src(4) | alpha_dst(4) | pad]
    with 768 B stride.  Layer-1 table is computed replicated (dense bf16
    matmuls); layer-2 table rows are produced per window pair by a fused dense
    matmul and AllGathered across the 8 cores (the AllGather hard-blocks the
    Pool queue in the cost model, so a single cch=1 collective is optimal).
  - Each core processes its ~106k incoming edges as 128-edge blocks (sorted by
    window, split into A/B streams at ASPLIT=25088 so gather indices fit
    int16).  SWDGE dma_gather fetches h[src] rows for gch=7 blocks per
    instruction; the steady state is gather-stream bound (back-to-back).
  - Attention: alpha_dst per edge via one-hot^T matmul (one-hots stored in
    fp8e4 -- PE matmuls take mixed bf16/fp8 operands); e = lrelu(as+ad) on
    DVE; exp broadcast to [4,64] on Activation; G' = G * exp multiplied IN
    PLACE in the gather tile (DVE 2x); segment sum + softmax denominator via
    PE matmuls accumulated per WINDOW PAIR in one PSUM tile
    [128, {seg0,seg1,den}, wi, 64].  No segment-max needed.
  - Pair finalize (flat [128,(half,wi,d)] layout, >=3-dim-AP verifier-safe):
    1/den via reciprocal + sel-matmul broadcast + one ACT copy; bias add from
    a pre-expanded [128,256] bias tile; ELU as min(exp(xb),1) + (max(xb,0)-1);
    h2/fc matmuls cover both windows in one 128-partition lhsT.  alpha_dst
    for layer 2 is stashed per pair via SBUF->SBUF DMA of the h2b ad columns
    and assembled with eyes-matmuls at end of layer 1; for layer 1 it comes
    from own-row gathers that fetch only the 256 B row tail (elem_step=EW).
  - Load pipeline: one-hot loads prefetched LOOKAHEAD chunks ahead, adP
    matmuls ADLA chunks ahead, CCPRE layer-2 load chunks streamed under the
    collective.  One Sigmoid at the very end ([128, 49] pair-major output).
"""

import sys

sys.path.insert(0, "/opt/trn_rl_repo")

from dataclasses import dataclass, field

import numpy as np
import ml_dtypes

BF = ml_dtypes.bfloat16
F8NP = ml_dtypes.float8_e4m3fn

import concourse.bass as bass
import concourse.bacc as bacc
import concourse.tile as tile
from concourse import mybir

F32 = mybir.dt.float32
BF16 = mybir.dt.bfloat16
F8 = mybir.dt.float8e4
I16 = mybir.dt.int16
OP = mybir.AluOpType
ACT = mybir.ActivationFunctionType

HEADS = 4
CH = 64
HC = 256
DIN = 128
NEG = 0.2
EW = 384                 # table row width in bf16 elems (768 B)
USED = 264               # used columns: h(256) + as(4) + ad(4)
AS_OFF = 256
AD_OFF = 260
ASPLIT = 25088           # first B-range table row (= chunk boundary for cch=2)
HISTART = 50176 - 32768  # = 17408, start row of the hi own-gather range
TUNE = dict(gt=6, gp=2, e4=6, oh=10, ot=10, er=5, fin=3, sm=8)
GQ = 16                  # blocks per dma_gather instruction
LOOKAHEAD = 3            # one-hot load prefetch depth (chunks, per side)
CCPRE = 12               # chunks of L2 loads streamed under the collective
ADLA = 2                 # adP matmul lookahead (chunks)
ES = 64                  # exp-broadcast columns computed on ACT (rest: DVE)
POOLS = dict(dx=3, dh=3, pa=3, pg=2, pp=3)


@dataclass
class Cfg:
    n_real: int = 50000
    nc: int = 8
    ndst: int = 64               # dst nodes per window
    wpc: int = 98                # windows per core
    gch: int = 7                 # blocks per chunk tile
    cch: int = 1                 # collective chunks for table2
    nA: list = field(default_factory=list)   # per-window A-block counts
    nB: list = field(default_factory=list)   # per-window B-block counts
    # kept for test.py compatibility (prints sumK)
    ka: list = field(default_factory=list)
    kb: list = field(default_factory=list)

    @property
    def pcn(self):
        return self.wpc * self.ndst          # nodes per core (6272)

    @property
    def rows(self):
        return self.nc * self.pcn            # table rows (50176)


def _pack_idx(blk):
    """blk: [nblk, 128] int16 -> wrapped-16 layout [128, nblk*8]."""
    nblk = blk.shape[0]
    pk = blk.reshape(nblk, 8, 16).transpose(2, 0, 1).reshape(16, nblk * 8)
    return np.ascontiguousarray(np.tile(pk, (8, 1)).astype(np.int16))


def build_layout(edge_index, cfg: Cfg):
    n = cfg.n_real
    NC, ND, W = cfg.nc, cfg.ndst, cfg.wpc
    src = np.asarray(edge_index[0], dtype=np.int64)
    dst = np.asarray(edge_index[1], dtype=np.int64)
    src = np.concatenate([src, np.arange(n, dtype=np.int64)])
    dst = np.concatenate([dst, np.arange(n, dtype=np.int64)])
    deg = np.bincount(dst, minlength=n)

    order = np.argsort(-deg, kind="stable")          # degree-descending
    node_of_row = np.full(cfg.rows, -1, np.int64)
    row_of_node = np.full(n, -1, np.int64)
    j = np.arange(512)
    r8 = j // 8
    c8 = j % 8
    core_j = np.where(r8 % 2 == 0, c8, 7 - c8)
    slot_j = r8
    CPW = W // cfg.cch                      # windows per collective chunk
    HPC = cfg.pcn // cfg.cch                # rows per core per chunk (3136)
    HALF = cfg.rows // cfg.cch              # rows per chunk (25088)
    for s in range(W):
        nodes = order[s * 512 : (s + 1) * 512]
        hh = s // CPW
        rows = (hh * HALF + core_j[: len(nodes)] * HPC
                + (s - hh * CPW) * ND + slot_j[: len(nodes)])
        node_of_row[rows] = nodes
        row_of_node[nodes] = rows

    drow = row_of_node[dst]
    hh_e = drow // HALF
    rem = drow % HALF
    core_e = rem // HPC
    loc2 = rem % HPC
    w_e = hh_e * CPW + loc2 // ND
    dloc = loc2 % ND
    srow = row_of_node[src]
    sideB = srow >= ASPLIT

    cntA = np.zeros((NC, W), np.int64)
    cntB = np.zeros((NC, W), np.int64)
    np.add.at(cntA, (core_e[~sideB], w_e[~sideB]), 1)
    np.add.at(cntB, (core_e[sideB], w_e[sideB]), 1)
    nA = np.maximum(1, np.ceil(cntA.max(axis=0) / 128).astype(np.int64))
    nB = np.maximum(1, np.ceil(cntB.max(axis=0) / 128).astype(np.int64))
    cfg.nA = nA.tolist()
    cfg.nB = nB.tolist()
    cfg.ka = nA.tolist()
    cfg.kb = nB.tolist()
    BA, BB = int(nA.sum()), int(nB.sum())
    offA = np.concatenate([[0], np.cumsum(nA)]).astype(int)
    offB = np.concatenate([[0], np.cumsum(nB)]).astype(int)

    eorder = np.lexsort((w_e, sideB, core_e))
    srow_s = srow[eorder]
    dloc_s = dloc[eorder]
    core_s = core_e[eorder]
    sideB_s = sideB[eorder]
    w_s = w_e[eorder]
    cstarts = np.searchsorted(core_s, np.arange(NC + 1))

    idx_cores, oh_cores, oht_cores = [], [], []
    lo_cores, hi_cores, m_lo, m_hi = [], [], [], []
    for c in range(NC):
        lo_, hi_ = cstarts[c], cstarts[c + 1]
        sr_c = srow_s[lo_:hi_]
        dl_c = dloc_s[lo_:hi_]
        sd_c = sideB_s[lo_:hi_]
        ww_c = w_s[lo_:hi_]
        idx_blk = np.zeros((BA + BB, 128), np.int16)
        dl_blk = np.full((BA + BB, 128), -1, np.int64)
        bstart = np.searchsorted(sd_c, 1)
        for sideflag, nW, off, base, elo, ehi in (
            (False, nA, offA, 0, 0, bstart),
            (True, nB, offB, BA, bstart, len(sr_c)),
        ):
            sr = sr_c[elo:ehi] - (ASPLIT if sideflag else 0)
            dl = dl_c[elo:ehi]
            ww = ww_c[elo:ehi]
            starts = np.searchsorted(ww, np.arange(W + 1))
            for w in range(W):
                s0, s1 = starts[w], starts[w + 1]
                cnt = s1 - s0
                b0 = base + off[w]
                fa = idx_blk[b0 : b0 + nW[w]].reshape(-1)
                fa[:cnt] = sr[s0:s1]
                fd = dl_blk[b0 : b0 + nW[w]].reshape(-1)
                fd[:cnt] = dl[s0:s1]
        idx_cores.append(_pack_idx(idx_blk))
        ohb = np.zeros((BA + BB, 128, ND), np.uint8)
        bb, pp = np.nonzero(dl_blk >= 0)
        ohb[bb, pp, dl_blk[bb, pp]] = 1
        oh_cores.append(np.ascontiguousarray(
            ohb.transpose(1, 0, 2).reshape(128, -1).astype(F8NP)))
        oht_cores.append(np.ascontiguousarray(
            ohb.transpose(2, 0, 1).reshape(ND, -1).astype(F8NP)))

        # own-row gather indices (for layer-1 alpha_dst): lo/hi + masks.
        # own position j = w*ND + d; row depends on the chunk-major layout.
        jj = np.arange(cfg.pcn)
        wn = jj // ND
        hh = wn // CPW
        own = hh * HALF + c * HPC + (wn - hh * CPW) * ND + (jj % ND)
        is_lo = own < ASPLIT
        lo_idx = np.where(is_lo, own, 0).astype(np.int16)
        hi_idx = np.where(~is_lo, own - HISTART, 0).astype(np.int16)
        lo_cores.append(_pack_idx(lo_idx.reshape(-1, 128)))
        hi_cores.append(_pack_idx(hi_idx.reshape(-1, 128)))
        # mask per position, laid out [partition, sub, head]
        ml = is_lo.astype(np.float32)
        ml4 = np.repeat(ml[:, None], HEADS, 1).reshape(-1, 128, HEADS)
        ml4 = ml4.transpose(1, 0, 2).reshape(128, -1)
        m_lo.append(ml4.astype(BF))
        m_hi.append((1.0 - ml4).astype(BF))

    return dict(
        node_of_row=node_of_row,
        row_of_node=row_of_node,
        idx=idx_cores, oh=oh_cores, oht=oht_cores,
        idxlo=lo_cores, idxhi=hi_cores, mlo=m_lo, mhi=m_hi,
        BA=BA, BB=BB,
    )


def _blkdiag(a):
    out = np.zeros((HC, HEADS), np.float32)
    a = np.asarray(a, np.float32)
    for h in range(HEADS):
        out[h * CH : (h + 1) * CH, h] = a[h]
    return out


def build_inputs(cfg: Cfg, layout, x, W1, a_src1, a_dst1, b1, W2, a_src2,
                 a_dst2, b2, fc_w, fc_b):
    node_of_row = layout["node_of_row"]
    xs = np.zeros((cfg.rows, DIN), np.float32)
    valid = node_of_row >= 0
    xs[valid] = np.asarray(x, np.float32)[node_of_row[valid]]
    xbf = np.ascontiguousarray(xs.T).astype(BF)            # [128, rows]

    W1 = np.asarray(W1, np.float32)
    W2 = np.asarray(W2, np.float32)
    w1aug = np.concatenate(
        [W1, W1 @ _blkdiag(a_src1), W1 @ _blkdiag(a_dst1)], axis=1).astype(BF)
    w2full = np.concatenate(
        [W2, W2 @ _blkdiag(a_src2), W2 @ _blkdiag(a_dst2)], axis=1).astype(BF)
    w2aug = np.ascontiguousarray(w2full.reshape(2, 128, USED))

    def _bexp(b):
        # [128, (half, wi, d)] pre-expanded bias
        bc = np.asarray(b, np.float32).reshape(2, 128).T    # [128, half]
        return np.ascontiguousarray(
            np.broadcast_to(bc[:, :, None, None], (128, 2, 2, 64))
            .reshape(128, 256))

    b1c = _bexp(b1)
    b2c = _bexp(b2)
    fcw = np.ascontiguousarray(
        np.asarray(fc_w, np.float32).reshape(2, 128, 1).astype(BF))
    fcb = np.full((128, 1), np.float32(np.asarray(fc_b).reshape(-1)[0]))

    sel = np.zeros((2, 4, 128), np.float32)
    for half in range(2):
        for h in range(2):
            sel[half, 2 * half + h, h * CH : (h + 1) * CH] = 1.0
    sel = np.ascontiguousarray(sel)

    eye = np.zeros((2, 128, 64), np.float32)
    eye[0, np.arange(64), np.arange(64)] = 1.0
    eye[1, 64 + np.arange(64), np.arange(64)] = 1.0
    eye = eye.astype(BF)

    base = dict(xbf=xbf, w1aug=w1aug, w2aug=w2aug, b1c=b1c, b2c=b2c,
                fcw=fcw, fcb=fcb, sel=sel, eye=eye)
    in_maps = []
    for c in range(cfg.nc):
        m = dict(base)
        m["idx"] = layout["idx"][c]
        m["oh"] = layout["oh"][c]
        m["oht"] = layout["oht"][c]
        m["idxlo"] = layout["idxlo"][c]
        m["idxhi"] = layout["idxhi"][c]
        m["mlo"] = layout["mlo"][c]
        m["mhi"] = layout["mhi"][c]
        in_maps.append(m)
    return in_maps


def build_program(cfg: Cfg, shared_out: bool = True):
    nc_b = bacc.Bacc(None, num_devices=cfg.nc)
    NC, ND, W, GCH = cfg.nc, cfg.ndst, cfg.wpc, cfg.gch
    nA, nB = cfg.nA, cfg.nB
    BA, BB = int(np.sum(nA)), int(np.sum(nB))
    NBLK = BA + BB
    ROWS = cfg.rows
    PCN = cfg.pcn
    NSUB = PCN // 128                                   # own-gather sub count
    offA = np.concatenate([[0], np.cumsum(nA)]).astype(int)
    offB = np.concatenate([[0], np.cumsum(nB)]).astype(int)

    xbfT = nc_b.dram_tensor("xbf", [DIN, ROWS], BF16, kind="ExternalInput")
    w1augT = nc_b.dram_tensor("w1aug", [DIN, USED], BF16, kind="ExternalInput")
    w2augT = nc_b.dram_tensor("w2aug", [2, 128, USED], BF16, kind="ExternalInput")
    b1cT = nc_b.dram_tensor("b1c", [128, 256], F32, kind="ExternalInput")
    b2cT = nc_b.dram_tensor("b2c", [128, 256], F32, kind="ExternalInput")
    fcwT = nc_b.dram_tensor("fcw", [2, 128, 1], BF16, kind="ExternalInput")
    fcbT = nc_b.dram_tensor("fcb", [128, 1], F32, kind="ExternalInput")
    selT = nc_b.dram_tensor("sel", [2, 4, 128], F32, kind="ExternalInput")
    idxT = nc_b.dram_tensor("idx", [128, NBLK * 8], I16, kind="ExternalInput")
    ohT = nc_b.dram_tensor("oh", [128, NBLK * ND], F8, kind="ExternalInput")
    ohtT = nc_b.dram_tensor("oht", [ND, NBLK * 128], F8, kind="ExternalInput")
    idxloT = nc_b.dram_tensor("idxlo", [128, NSUB * 8], I16, kind="ExternalInput")
    idxhiT = nc_b.dram_tensor("idxhi", [128, NSUB * 8], I16, kind="ExternalInput")
    mloT = nc_b.dram_tensor("mlo", [128, NSUB * HEADS], BF16, kind="ExternalInput")
    mhiT = nc_b.dram_tensor("mhi", [128, NSUB * HEADS], BF16, kind="ExternalInput")
    eyeT = nc_b.dram_tensor("eye", [2, 128, 64], BF16, kind="ExternalInput")
    yT = nc_b.dram_tensor("y", [128, W // 2], F32, kind="ExternalOutput")

    HPCg = PCN // cfg.cch
    HALFg = ROWS // cfg.cch
    table1 = nc_b.dram_tensor("table1", [ROWS, EW], BF16)
    tab2own_t = [
        nc_b.dram_tensor(f"tab2own{k}", [HPCg, EW], BF16)
        for k in range(cfg.cch)
    ]
    table2_t = [
        nc_b.dram_tensor(
            f"table2_{k}", [HALFg, EW], BF16,
            addr_space="Shared" if shared_out else "Local")
        for k in range(cfg.cch)
    ]

    # chunk plan over the A-stream then B-stream of blocks
    chunks = []
    for base, nb in ((0, BA), (BA, BB)):
        b = 0
        while b < nb:
            wdt = min(GCH, nb - b)
            chunks.append((base + b, wdt, base == BA))
            b += wdt
    chunk_of_blk = {}
    for ci, (b0, cw, _) in enumerate(chunks):
        for k in range(cw):
            chunk_of_blk[b0 + k] = (ci, k)

    win_of_blk = np.zeros(NBLK, int)
    for w in range(W):
        win_of_blk[offA[w] : offA[w + 1]] = w
        win_of_blk[BA + offB[w] : BA + offB[w + 1]] = w

    CPW = W // cfg.cch

    import contextlib

    with tile.TileContext(nc_b) as tc:
        ctx = [
            tc.tile_pool(name="cst", bufs=1),
            tc.tile_pool(name="dx", bufs=POOLS["dx"]),
            tc.tile_pool(name="dh", bufs=POOLS["dh"]),
            tc.tile_pool(name="ixp", bufs=3),
            tc.tile_pool(name="gtA", bufs=TUNE["gt"]),
            tc.tile_pool(name="gtB", bufs=TUNE["gt"]),
            tc.tile_pool(name="er", bufs=TUNE["er"]),
            tc.tile_pool(name="e4A", bufs=TUNE["e4"]),
            tc.tile_pool(name="e4B", bufs=TUNE["e4"]),
            tc.tile_pool(name="og", bufs=2),
            tc.tile_pool(name="ohA", bufs=TUNE["oh"]),
            tc.tile_pool(name="ohB", bufs=TUNE["oh"]),
            tc.tile_pool(name="otA", bufs=TUNE["ot"]),
            tc.tile_pool(name="otB", bufs=TUNE["ot"]),
            tc.tile_pool(name="sm", bufs=TUNE["sm"]),
            tc.tile_pool(name="fin", bufs=TUNE["fin"]),
            tc.tile_pool(name="sp", bufs=1),
            tc.tile_pool(name="pa", bufs=POOLS["pa"], space="PSUM"),
            tc.tile_pool(name="pg", bufs=POOLS["pg"], space="PSUM"),
            tc.tile_pool(name="pp", bufs=POOLS["pp"], space="PSUM"),
        ]
        with contextlib.ExitStack() as st:
            (cst, dx, dh, ixp, gtA, gtB, er, e4A, e4B, og,
             ohA, ohB, otA, otB, sm, fin, sp,
             pa, pg, pp) = [st.enter_context(m) for m in ctx]

            # ---- constants ----
            w1s = cst.tile([128, USED], BF16)
            nc_b.sync.dma_start(out=w1s[:], in_=w1augT[:, :])
            w2s = cst.tile([128, 2, USED], BF16)
            nc_b.sync.dma_start(out=w2s[:, 0, :], in_=w2augT[0, :, :])
            nc_b.sync.dma_start(out=w2s[:, 1, :], in_=w2augT[1, :, :])
            b1s = cst.tile([128, 256], F32)
            nc_b.sync.dma_start(out=b1s[:], in_=b1cT[:, :])
            b2s = cst.tile([128, 256], F32)
            nc_b.sync.dma_start(out=b2s[:], in_=b2cT[:, :])
            fcws = cst.tile([128, 2, 1], BF16)
            nc_b.sync.dma_start(out=fcws[:, 0, :], in_=fcwT[0, :, :])
            nc_b.sync.dma_start(out=fcws[:, 1, :], in_=fcwT[1, :, :])
            fcbs = cst.tile([128, 1], F32)
            nc_b.sync.dma_start(out=fcbs[:], in_=fcbT[:, :])
            sels = cst.tile([4, 2, 128], F32)
            nc_b.sync.dma_start(out=sels[:, 0, :], in_=selT[0, :, :])
            nc_b.sync.dma_start(out=sels[:, 1, :], in_=selT[1, :, :])
            ixs = cst.tile([128, NBLK * 8], I16)
            nc_b.sync.dma_start(out=ixs[:], in_=idxT[:, :])
            ixlo = cst.tile([128, NSUB * 8], I16)
            nc_b.sync.dma_start(out=ixlo[:], in_=idxloT[:, :])
            ixhi = cst.tile([128, NSUB * 8], I16)
            nc_b.sync.dma_start(out=ixhi[:], in_=idxhiT[:, :])
            mlos = cst.tile([128, NSUB * HEADS], BF16)
            nc_b.sync.dma_start(out=mlos[:], in_=mloT[:, :])
            mhis = cst.tile([128, NSUB * HEADS], BF16)
            nc_b.sync.dma_start(out=mhis[:], in_=mhiT[:, :])
            eyes = cst.tile([128, 2, 64], BF16)
            nc_b.sync.dma_start(out=eyes[:, 0, :], in_=eyeT[0, :, :])
            nc_b.sync.dma_start(out=eyes[:, 1, :], in_=eyeT[1, :, :])
            adW1 = cst.tile([ND, W, HEADS], BF16)
            adW2 = cst.tile([ND, W, HEADS], BF16)
            zAll = cst.tile([128, W // 2], F32)
            negones = cst.tile([128, 1], F32)
            nc_b.vector.memset(negones[:], -1.0)
            adraw2 = cst.tile([128, W // 2, HEADS], BF16)

            # ---- dense phase (replicated): table1 rows = [x @ W1aug] ----
            NT8 = ROWS // 1024
            for t8 in range(NT8):
                xin = dx.tile([128, 8, 128], BF16, tag="xin")
                nc_b.sync.dma_start(
                    out=xin[:], in_=xbfT[:, t8 * 1024 : (t8 + 1) * 1024])
                hb = dh.tile([128, 8, USED], BF16, tag="hb")
                for i in range(8):
                    pht = pp.tile([128, USED], F32, space="PSUM", tag="pp")
                    nc_b.tensor.matmul(
                        out=pht[:], lhsT=xin[:, i, :], rhs=w1s[:],
                        start=True, stop=True)
                    if i % 2 == 0:
                        nc_b.vector.tensor_copy(out=hb[:, i, :], in_=pht[:])
                    else:
                        nc_b.scalar.copy(out=hb[:, i, :], in_=pht[:])
                nc_b.sync.dma_start(
                    out=bass.AP(
                        tensor=table1[:, :].tensor,
                        offset=t8 * 1024 * EW,
                        ap=[[EW, 128], [EW * 128, 8], [1, USED]],
                    ),
                    in_=bass.AP(
                        tensor=hb[:].tensor, offset=hb[:].offset,
                        ap=[hb[:].ap[0], [USED, 8], [1, USED]],
                    ),
                )

            # ---- own-row ad gather (layer 1), lo/hi + mask blend ----
            # gather only the 256 B row tail [512:768) that holds the ad
            # columns (ad sits at local elem offset AD_OFF-256)
            OEL = 128                                   # elems gathered/row
            ADL = AD_OFF - 256                          # local ad offset
            adraw = cst.tile([128, NSUB, HEADS], BF16)
            OGC = 8
            for s0 in range(0, NSUB, OGC):
                sw = min(OGC, NSUB - s0)
                parts = []
                for rng, ixt, mt in ((table1[0:ASPLIT, :], ixlo, mlos),
                                     (table1[HISTART:ROWS, :], ixhi, mhis)):
                    gg = og.tile([128, OGC, OEL], BF16, tag="og")
                    nc_b.gpsimd.dma_gather(
                        out_ap=bass.AP(
                            tensor=gg[:].tensor, offset=gg[:].offset,
                            ap=[gg[:].ap[0], [OEL, sw], [1, OEL]],
                        ),
                        in_ap=bass.AP(
                            tensor=rng.tensor, offset=rng.offset + 256,
                            ap=[[EW, rng.ap[0][1]], [1, OEL]],
                        ),
                        idxs_ap=ixt[:, s0 * 8 : (s0 + sw) * 8],
                        num_idxs=128 * sw, num_idxs_reg=128 * sw,
                        elem_size=OEL, elem_step=EW)
                    tt = og.tile([128, OGC, HEADS], BF16, tag="ogm")
                    nc_b.vector.tensor_tensor(
                        out=tt[:, 0:sw, :],
                        in0=bass.AP(tensor=gg[:].tensor,
                                    offset=gg[:].offset + ADL,
                                    ap=[gg[:].ap[0], [OEL, sw], [1, HEADS]]),
                        in1=mt[:, s0 * HEADS : (s0 + sw) * HEADS],
                        op=OP.mult)
                    parts.append(tt)
                nc_b.vector.tensor_tensor(
                    out=adraw[:, s0 : s0 + sw, :],
                    in0=parts[0][:, 0:sw, :], in1=parts[1][:, 0:sw, :],
                    op=OP.add)
            for par in range(2):
                adT = pa.tile([128, 196], F32, space="PSUM", tag="ad")
                adPP = adT[0:ND, 0 : NSUB * HEADS]
                nc_b.tensor.matmul(
                    out=adPP, lhsT=eyes[:, par, :], rhs=adraw[:],
                    start=True, stop=True)
                nc_b.vector.tensor_copy(
                    out=bass.AP(
                        tensor=adW1[:].tensor,
                        offset=adW1[:].offset + par * HEADS,
                        ap=[adW1[:].ap[0], [2 * HEADS, NSUB], [1, HEADS]],
                    ),
                    in_=adPP)

            def cc_chunk(k):
                nc_b.gpsimd.collective_compute(
                    "AllGather",
                    OP.bypass,
                    replica_groups=[list(range(NC))],
                    ins=[tab2own_t[k][:, :].opt()],
                    outs=[table2_t[k][:, :].opt()],
                )

            def mk_state(layer):
                st = dict(
                    emitted=[False] * len(chunks),
                    preloaded=[None] * len(chunks),
                    preadp=[None] * len(chunks),
                    ctiles=[None] * len(chunks),
                )

                def emit_loads(ci):
                    # SP-only part: prefetchable many chunks ahead
                    if st["preloaded"][ci] is not None:
                        return st["preloaded"][ci]
                    b0, cw, is_b = chunks[ci]
                    oh_p = ohB if is_b else ohA
                    ot_p = otB if is_b else otA
                    oh_t = oh_p.tile([128, GCH * ND], F8, tag="oh")
                    nc_b.sync.dma_start(
                        out=oh_t[:, 0 : cw * ND],
                        in_=ohT[:, b0 * ND : (b0 + cw) * ND])
                    oht_t = ot_p.tile([ND, GCH * 128], F8, tag="oht")
                    nc_b.sync.dma_start(
                        out=oht_t[:, 0 : cw * 128],
                        in_=ohtT[:, b0 * 128 : (b0 + cw) * 128])
                    st["preloaded"][ci] = (oh_t, oht_t)
                    return st["preloaded"][ci]

                def emit_adp(ci):
                    # PE part, done near emit_chunk time (bounded PSUM use)
                    if st["preadp"][ci] is not None:
                        return st["preadp"][ci]
                    b0, cw, is_b = chunks[ci]
                    adW = adW1 if layer == 1 else adW2
                    oh_t, oht_t = emit_loads(ci)
                    adT = pa.tile([128, 196], F32, space="PSUM", tag="ad")
                    adP = adT[:, 0 : GCH * HEADS]
                    for k in range(cw):
                        wv = int(win_of_blk[b0 + k])
                        nc_b.tensor.matmul(
                            out=adP[:, k * HEADS : (k + 1) * HEADS],
                            lhsT=oht_t[:, k * 128 : (k + 1) * 128],
                            rhs=adW[:, wv, :],
                            start=True, stop=True)
                    st["preadp"][ci] = (oh_t, adP)
                    return st["preadp"][ci]

                st["emit_loads"] = emit_loads
                st["emit_adp"] = emit_adp
                return st

            # chunk index ranges per side (for load lookahead)
            nA_chunks = len([c for c in chunks if not c[2]])

            def edge_layer(layer, st, pre_cc=None):
                emitted = st["emitted"]
                ctiles = st["ctiles"]
                emit_loads = st["emit_loads"]
                emit_adp = st["emit_adp"]

                def emit_chunk(ci):
                    b0, cw, is_b = chunks[ci]
                    gt_p = gtB if is_b else gtA
                    e4_p = e4B if is_b else e4A
                    # prefetch loads for upcoming chunks of this side
                    lo, hi = ((nA_chunks, len(chunks)) if is_b
                              else (0, nA_chunks))
                    for cj in range(ci, min(ci + LOOKAHEAD, hi)):
                        emit_loads(cj)
                    oh_t, adP = emit_adp(ci)
                    for cj in range(ci + 1, min(ci + ADLA + 1, hi)):
                        emit_adp(cj)
                    g = gt_p.tile([128, GCH, EW], BF16, tag="g")
                    if layer == 1:
                        tab = table1
                    elif cfg.cch == 1:
                        tab = table2_t[0]
                    else:
                        assert ASPLIT == HALFg
                        tab = table2_t[1 if is_b else 0]
                    if layer == 1 or cfg.cch == 1:
                        in_ap = (tab[ASPLIT:ROWS, :] if is_b
                                 else tab[0:ASPLIT, :])
                    else:
                        in_ap = tab[:, :]
                    for q0 in range(0, cw, GQ):
                        qw = min(GQ, cw - q0)
                        nc_b.gpsimd.dma_gather(
                            out_ap=bass.AP(
                                tensor=g[:].tensor,
                                offset=g[:].offset + q0 * EW,
                                ap=[g[:].ap[0], [EW, qw], [1, EW]],
                            ),
                            in_ap=in_ap,
                            idxs_ap=ixs[:, (b0 + q0) * 8 : (b0 + q0 + qw) * 8],
                            num_idxs=128 * qw,
                            num_idxs_reg=128 * qw,
                            elem_size=EW,
                        )
                    e0 = sm.tile([128, GCH * HEADS], BF16, tag="e0")
                    nc_b.vector.tensor_tensor(
                        out=e0[:, 0 : cw * HEADS],
                        in0=bass.AP(
                            tensor=g[:].tensor, offset=g[:].offset + AS_OFF,
                            ap=[g[:].ap[0], [EW, cw], [1, HEADS]],
                        ),
                        in1=adP[:, 0 : cw * HEADS],
                        op=OP.add)
                    e2 = sm.tile([128, GCH * HEADS], BF16, tag="e2")
                    nc_b.vector.scalar_tensor_tensor(
                        out=e2[:, 0 : cw * HEADS], in0=e0[:, 0 : cw * HEADS],
                        scalar=NEG, in1=e0[:, 0 : cw * HEADS],
                        op0=OP.mult, op1=OP.max)
                    # exp broadcast split: ACT fills cols [0:ES), DVE
                    # replicates the rest with a 4x-mode copy
                    ert = er.tile([128, GCH, HEADS, CH], BF16, tag="er")
                    nc_b.scalar.activation(
                        out=bass.AP(
                            tensor=ert[:].tensor, offset=ert[:].offset,
                            ap=[ert[:].ap[0], [CH, cw * HEADS], [1, ES]],
                        ),
                        in_=bass.AP(
                            tensor=e2[:].tensor, offset=e2[:].offset,
                            ap=[e2[:].ap[0], [1, cw * HEADS], [0, ES]],
                        ),
                        func=ACT.Exp)
                    if ES < CH:
                        nc_b.vector.tensor_copy(
                            out=bass.AP(
                                tensor=ert[:].tensor,
                                offset=ert[:].offset + ES,
                                ap=[ert[:].ap[0], [CH, cw * HEADS],
                                    [1, CH - ES]],
                            ),
                            in_=bass.AP(
                                tensor=ert[:].tensor, offset=ert[:].offset,
                                ap=[ert[:].ap[0], [CH, cw * HEADS],
                                    [1, CH - ES]],
                            ))
                    # scale h in place inside the gather tile (saves SBUF)
                    gpt = g
                    nc_b.vector.tensor_tensor(
                        out=bass.AP(
                            tensor=g[:].tensor, offset=g[:].offset,
                            ap=[g[:].ap[0], [EW, cw], [1, HC]],
                        ),
                        in0=bass.AP(
                            tensor=g[:].tensor, offset=g[:].offset,
                            ap=[g[:].ap[0], [EW, cw], [1, HC]],
                        ),
                        in1=ert[:, 0:cw, :, :],
                        op=OP.mult)
                    exp4 = e4_p.tile([128, GCH, HEADS], BF16, tag="e4")
                    nc_b.vector.tensor_copy(
                        out=exp4[:, 0:cw, :],
                        in_=bass.AP(
                            tensor=ert[:].tensor, offset=ert[:].offset,
                            ap=[ert[:].ap[0], [HEADS * CH, cw], [CH, HEADS]],
                        ))
                    emitted[ci] = True
                    ctiles[ci] = (gpt, exp4, oh_t)

                # windows processed in PAIRS: one PSUM tile + one finalize
                # chain per two windows (layout [128, wi, {seg0,seg1,den}, ND])
                for w in range(0, W, 2):
                    # PSUM layout [128, {seg_half0, seg_half1, den}, wi, ND]
                    segF = pg.tile([128, 3, 2, ND], F32, space="PSUM",
                                   tag="seg")
                    first_mm = True
                    lasts = []
                    for wi in range(2):
                        bl = list(range(offA[w + wi], offA[w + wi + 1])) + \
                             list(range(BA + offB[w + wi],
                                        BA + offB[w + wi + 1]))
                        lasts.append(bl[-1])
                    for wi in range(2):
                        bl = list(range(offA[w + wi], offA[w + wi + 1])) + \
                             list(range(BA + offB[w + wi],
                                        BA + offB[w + wi + 1]))
                        for b in bl:
                            ci, k = chunk_of_blk[b]
                            if not emitted[ci]:
                                emit_chunk(ci)
                            gpt, exp4, oh_t = ctiles[ci]
                            for half in range(2):
                                nc_b.tensor.matmul(
                                    out=segF[:, half, wi, :],
                                    lhsT=bass.AP(
                                        tensor=gpt[:].tensor,
                                        offset=(gpt[:].offset + k * EW
                                                + half * 128),
                                        ap=[gpt[:].ap[0], [1, 128]],
                                    ),
                                    rhs=oh_t[:, k * ND : (k + 1) * ND],
                                    start=first_mm, stop=False)
                                first_mm = False
                            nc_b.tensor.matmul(
                                out=segF[0:4, 2, wi, :],
                                lhsT=exp4[:, k, :],
                                rhs=oh_t[:, k * ND : (k + 1) * ND],
                                start=False,
                                stop=(wi == 1 and b == lasts[1]))

                    # ---- finalize window pair (w, w+1) ----
                    den_s = fin.tile([4, 2, ND], F32, tag="dens")
                    nc_b.vector.reciprocal(
                        out=den_s[:], in_=segF[0:4, 2, :, :])
                    rdT = pa.tile([128, 2, 2, ND], F32, space="PSUM",
                                  tag="ad")
                    for half in range(2):
                        nc_b.tensor.matmul(
                            out=rdT[:, half, :, :],
                            lhsT=sels[:, half, :],
                            rhs=den_s[:],
                            start=True, stop=True)
                    # flat [128, (wi, half, d)] layout for finalize tensors
                    rdenS = fin.tile([128, 2 * 2 * ND], F32, tag="rdenS")
                    nc_b.scalar.copy(
                        out=rdenS[:],
                        in_=bass.AP(
                            tensor=rdT[:].tensor, offset=rdT[:].offset,
                            ap=[rdT[:].ap[0], [1, 2 * 2 * ND]],
                        ))
                    nrm = fin.tile([128, 2 * 2 * ND], F32, tag="nrm")
                    nc_b.vector.tensor_tensor(
                        out=nrm[:],
                        in0=bass.AP(
                            tensor=segF[:].tensor, offset=segF[:].offset,
                            ap=[segF[:].ap[0], [1, 2 * 2 * ND]],
                        ),
                        in1=rdenS[:], op=OP.mult)
                    bsel = b1s if layer == 1 else b2s
                    xb = fin.tile([128, 2 * 2 * ND], F32, tag="xb")
                    nc_b.vector.tensor_tensor(
                        out=xb[:], in0=nrm[:], in1=bsel[:], op=OP.add)
                    em = fin.tile([128, 2 * 2 * ND], F32, tag="em")
                    nc_b.scalar.activation(out=em[:], in_=xb[:], func=ACT.Exp)
                    rl = fin.tile([128, 2 * 2 * ND], F32, tag="rl")
                    nc_b.vector.scalar_tensor_tensor(
                        out=rl[:], in0=xb[:], scalar=0.0,
                        in1=bass.AP(
                            tensor=negones[:].tensor,
                            offset=negones[:].offset,
                            ap=[negones[:].ap[0], [0, 2 * 2 * ND]],
                        ),
                        op0=OP.max, op1=OP.add)
                    eluT = fin.tile([128, 2, 2, ND], BF16, tag="eluT")
                    nc_b.vector.scalar_tensor_tensor(
                        out=bass.AP(
                            tensor=eluT[:].tensor, offset=eluT[:].offset,
                            ap=[eluT[:].ap[0], [1, 2 * 2 * ND]],
                        ),
                        in0=em[:], scalar=1.0, in1=rl[:],
                        op0=OP.min, op1=OP.add)
                    ppt = pp.tile([128, USED], F32, space="PSUM", tag="pp")
                    if layer == 1:
                        h2P = ppt[:, :]
                        for half in range(2):
                            nc_b.tensor.matmul(
                                out=h2P,
                                lhsT=eluT[:, half, :, :],
                                rhs=w2s[:, half, :],
                                start=(half == 0), stop=(half == 1))
                        h2b = fin.tile([128, EW], BF16, tag="h2b")
                        nc_b.scalar.copy(out=h2b[:, 0:USED], in_=h2P)
                        # stash this pair's alpha_dst rows ([wi,d] partitions)
                        nc_b.sync.dma_start(
                            out=adraw2[:, w // 2, :],
                            in_=h2b[:, AD_OFF:USED])
                        nc_b.sync.dma_start(
                            out=tab2own_t[0][w * ND : (w + 2) * ND, :],
                            in_=h2b[:])
                        if w + 2 == W:
                            for par in range(2):
                                adT2 = pa.tile([128, 196], F32,
                                               space="PSUM", tag="ad")
                                adPP2 = adT2[0:ND, 0 : (W // 2) * HEADS]
                                nc_b.tensor.matmul(
                                    out=adPP2, lhsT=eyes[:, par, :],
                                    rhs=adraw2[:],
                                    start=True, stop=True)
                                nc_b.vector.tensor_copy(
                                    out=bass.AP(
                                        tensor=adW2[:].tensor,
                                        offset=(adW2[:].offset
                                                + par * HEADS),
                                        ap=[adW2[:].ap[0],
                                            [2 * HEADS, W // 2],
                                            [1, HEADS]],
                                    ),
                                    in_=adPP2)
                            if pre_cc is not None:
                                pre_cc()
                            cc_chunk(0)
                    else:
                        zP = ppt[:, 0:1]
                        for half in range(2):
                            nc_b.tensor.matmul(
                                out=zP,
                                lhsT=eluT[:, half, :, :],
                                rhs=fcws[:, half, :],
                                start=(half == 0), stop=(half == 1))
                        nc_b.vector.tensor_copy(
                            out=zAll[:, w // 2 : w // 2 + 1], in_=zP)

            marks = {}
            nc_b._phase_marks = marks
            st1 = mk_state(1)
            st2 = mk_state(2)
            firstB = len([c for c in chunks if not c[2]])

            def prefetch2():
                # stream layer-2 one-hot loads under the collective
                for ci in range(min(CCPRE, firstB)):
                    st2["emit_loads"](ci)
                for ci in range(firstB, min(firstB + CCPRE, len(chunks))):
                    st2["emit_loads"](ci)

            marks["setup_end"] = len(nc_b.inst_map)
            edge_layer(1, st1, pre_cc=prefetch2)
            marks["e1_end"] = len(nc_b.inst_map)
            edge_layer(2, st2)
            marks["e2_end"] = len(nc_b.inst_map)

            ysig = cst.tile([128, W // 2], F32)
            nc_b.scalar.activation(
                out=ysig[:], in_=zAll[:], func=ACT.Sigmoid,
                bias=fcbs[:, 0:1], scale=1.0)
            nc_b.sync.dma_start(out=yT[:, :], in_=ysig[:])

    nc_b.finalize()
    return nc_b


def assemble_output(cfg: Cfg, layout, results):
    node_of_row = layout["node_of_row"]
    yfull = np.zeros((cfg.n_real, 1), np.float32)
    CPW = cfg.wpc // cfg.cch
    HPC = cfg.pcn // cfg.cch
    HALF = cfg.rows // cfg.cch
    for c in range(cfg.nc):
        yc = np.asarray(results[c]["y"])               # [128, 49]
        q, p = np.meshgrid(np.arange(128), np.arange(cfg.wpc // 2),
                           indexing="ij")
        w = 2 * p + q // cfg.ndst
        d = q % cfg.ndst
        hh = w // CPW
        rows = hh * HALF + c * HPC + (w - hh * CPW) * cfg.ndst + d
        nodes = node_of_row[rows.reshape(-1)]
        ok = nodes >= 0
        yfull[nodes[ok], 0] = yc.reshape(-1)[ok]
    return yfull


def _absorb_device_wedge():
    """Run a trivial 8-core kernel; a crashed prior session can leave the
    NeuronCores in NRT_EXEC_UNIT_UNRECOVERABLE state, which a fresh trivial
    execution clears."""
    try:
        from concourse.bass_utils import run_bass_kernel_spmd

        nc_t = bacc.Bacc(None, num_devices=8)
        a = nc_t.dram_tensor("a", [128, 128], F32, kind="ExternalInput")
        o = nc_t.dram_tensor("o", [128, 128], F32, kind="ExternalOutput")
        with tile.TileContext(nc_t) as tc:
            with tc.tile_pool(name="sb", bufs=1) as sb:
                t = sb.tile([128, 128], F32)
                nc_t.sync.dma_start(out=t[:], in_=a[:, :])
                nc_t.sync.dma_start(out=o[:, :], in_=t[:])
        nc_t.finalize()
        run_bass_kernel_spmd(
            nc_t, [{"a": np.zeros((128, 128), np.float32)}] * 8,
            core_ids=list(range(8)),
        )
    except Exception:
        pass


def kernel(**inputs):
    from concourse.bass_utils import run_bass_kernel_spmd

    cfg = Cfg()
    layout = build_layout(inputs["edge_index"], cfg)
    in_maps = build_inputs(
        cfg, layout,
        inputs["x"], inputs["W1"], inputs["a_src1"], inputs["a_dst1"],
        inputs["b1"], inputs["W2"], inputs["a_src2"], inputs["a_dst2"],
        inputs["b2"], inputs["fc_w"], inputs["fc_b"],
    )
    nc_b = build_program(cfg, shared_out=True)
    last_err = None
    for attempt in range(3):
        try:
            res = run_bass_kernel_spmd(
                nc_b, in_maps, core_ids=list(range(cfg.nc)))
            return assemble_output(cfg, layout, res.results)
        except Exception as e:  # wedged device from a prior crashed session
            last_err = e
            _absorb_device_wedge()
    raise last_err


if __name__ == "__main__":
    pass



# revision 104
# speedup vs baseline: 1.0076x; 1.0076x over previous
"""Two-layer GAT (PyG GATConv-style) on 8 Trainium2 NeuronCores via Bass/Tile.

Edges-on-partitions design (v3, pair-window pipeline):
  - Nodes are degree-stratified into 98 strata of 512; each stratum contributes
    64 nodes to every core (snake order), giving core-major table rows
    row = core*6272 + window*64 + slot.  Window = 64 destination nodes.
  - A bf16 feature table holds rows [h(256) | alpha_src(4) | alpha_dst(4) | pad]
    with 768 B stride.  Layer-1 table is computed replicated (dense bf16
    matmuls); layer-2 table rows are produced per window pair by a fused dense
    matmul and AllGathered across the 8 cores (the AllGather hard-blocks the
    Pool queue in the cost model, so a single cch=1 collective is optimal).
  - Each core processes its ~106k incoming edges as 128-edge blocks (sorted by
    window, split into A/B streams at ASPLIT=25088 so gather indices fit
    int16).  SWDGE dma_gather fetches h[src] rows for gch=7 blocks per
    instruction; the steady state is gather-stream bound (back-to-back).
  - Attention: alpha_dst per edge via one-hot^T matmul (one-hots stored in
    fp8e4 -- PE matmuls take mixed bf16/fp8 operands); e = lrelu(as+ad) on
    DVE; exp broadcast to [4,64] on Activation; G' = G * exp multiplied IN
    PLACE in the gather tile (DVE 2x); segment sum + softmax denominator via
    PE matmuls accumulated per WINDOW PAIR in one PSUM tile
    [128, {seg0,seg1,den}, wi, 64].  No segment-max needed.
  - Pair finalize (flat [128,(half,wi,d)] layout, >=3-dim-AP verifier-safe):
    1/den via reciprocal + sel-matmul broadcast + one ACT copy; bias add from
    a pre-expanded [128,256] bias tile; ELU as min(exp(xb),1) + (max(xb,0)-1);
    h2/fc matmuls cover both windows in one 128-partition lhsT.  alpha_dst
    for layer 2 is stashed per pair via SBUF->SBUF DMA of the h2b ad columns
    and assembled with eyes-matmuls at end of layer 1; for layer 1 it comes
    from own-row gathers that fetch only the 256 B row tail (elem_step=EW).
  - Load pipeline: one-hot loads prefetched LOOKAHEAD chunks ahead, adP
    matmuls ADLA chunks ahead, CCPRE layer-2 load chunks streamed under the
    collective.  One Sigmoid at the very end ([128, 49] pair-major output).
"""

import sys

sys.path.insert(0, "/opt/trn_rl_repo")

from dataclasses import dataclass, field

import numpy as np
import ml_dtypes

BF = ml_dtypes.bfloat16
F8NP = ml_dtypes.float8_e4m3fn

import concourse.bass as bass
import concourse.bacc as bacc
import concourse.tile as tile
from concourse import mybir

F32 = mybir.dt.float32
BF16 = mybir.dt.bfloat16
F8 = mybir.dt.float8e4
I16 = mybir.dt.int16
OP = mybir.AluOpType
ACT = mybir.ActivationFunctionType

HEADS = 4
CH = 64
HC = 256
DIN = 128
NEG = 0.2
EW = 384                 # table row width in bf16 elems (768 B)
USED = 264               # used columns: h(256) + as(4) + ad(4)
AS_OFF = 256
AD_OFF = 260
ASPLIT = 25088           # first B-range table row (= chunk boundary for cch=2)
HISTART = 50176 - 32768  # = 17408, start row of the hi own-gather range
TUNE = dict(gt=6, gp=2, e4=6, oh=10, ot=10, er=5, fin=3, sm=8)
GQ = 16                  # blocks per dma_gather instruction
LOOKAHEAD = 3            # one-hot load prefetch depth (chunks, per side)
CCPRE = 12               # chunks of L2 loads streamed under the collective
ADLA = 2                 # adP matmul lookahead (chunks)
ES = 64                  # exp-broadcast columns computed on ACT (rest: DVE)
POOLS = dict(dx=3, dh=3, pa=3, pg=2, pp=3)


@dataclass
class Cfg:
    n_real: int = 50000
    nc: int = 8
    ndst: int = 64               # dst nodes per window
    wpc: int = 98                # windows per core
    gch: int = 7                 # blocks per chunk tile
    cch: int = 1                 # collective chunks for table2
    nA: list = field(default_factory=list)   # per-window A-block counts
    nB: list = field(default_factory=list)   # per-window B-block counts
    # kept for test.py compatibility (prints sumK)
    ka: list = field(default_factory=list)
    kb: list = field(default_factory=list)

    @property
    def pcn(self):
        return self.wpc * self.ndst          # nodes per core (6272)

    @property
    def rows(self):
        return self.nc * self.pcn            # table rows (50176)


def _pack_idx(blk):
    """blk: [nblk, 128] int16 -> wrapped-16 layout [128, nblk*8]."""
    nblk = blk.shape[0]
    pk = blk.reshape(nblk, 8, 16).transpose(2, 0, 1).reshape(16, nblk * 8)
    return np.ascontiguousarray(np.tile(pk, (8, 1)).astype(np.int16))


def build_layout(edge_index, cfg: Cfg):
    n = cfg.n_real
    NC, ND, W = cfg.nc, cfg.ndst, cfg.wpc
    src = np.asarray(edge_index[0], dtype=np.int64)
    dst = np.asarray(edge_index[1], dtype=np.int64)
    src = np.concatenate([src, np.arange(n, dtype=np.int64)])
    dst = np.concatenate([dst, np.arange(n, dtype=np.int64)])
    deg = np.bincount(dst, minlength=n)

    order = np.argsort(-deg, kind="stable")          # degree-descending
    node_of_row = np.full(cfg.rows, -1, np.int64)
    row_of_node = np.full(n, -1, np.int64)
    j = np.arange(512)
    r8 = j // 8
    c8 = j % 8
    core_j = np.where(r8 % 2 == 0, c8, 7 - c8)
    slot_j = r8
    CPW = W // cfg.cch                      # windows per collective chunk
    HPC = cfg.pcn // cfg.cch                # rows per core per chunk (3136)
    HALF = cfg.rows // cfg.cch              # rows per chunk (25088)
    for s in range(W):
        nodes = order[s * 512 : (s + 1) * 512]
        hh = s // CPW
        rows = (hh * HALF + core_j[: len(nodes)] * HPC
                + (s - hh * CPW) * ND + slot_j[: len(nodes)])
        node_of_row[rows] = nodes
        row_of_node[nodes] = rows

    drow = row_of_node[dst]
    hh_e = drow // HALF
    rem = drow % HALF
    core_e = rem // HPC
    loc2 = rem % HPC
    w_e = hh_e * CPW + loc2 // ND
    dloc = loc2 % ND
    srow = row_of_node[src]
    sideB = srow >= ASPLIT

    cntA = np.zeros((NC, W), np.int64)
    cntB = np.zeros((NC, W), np.int64)
    np.add.at(cntA, (core_e[~sideB], w_e[~sideB]), 1)
    np.add.at(cntB, (core_e[sideB], w_e[sideB]), 1)
    nA = np.maximum(1, np.ceil(cntA.max(axis=0) / 128).astype(np.int64))
    nB = np.maximum(1, np.ceil(cntB.max(axis=0) / 128).astype(np.int64))
    cfg.nA = nA.tolist()
    cfg.nB = nB.tolist()
    cfg.ka = nA.tolist()
    cfg.kb = nB.tolist()
    BA, BB = int(nA.sum()), int(nB.sum())
    offA = np.concatenate([[0], np.cumsum(nA)]).astype(int)
    offB = np.concatenate([[0], np.cumsum(nB)]).astype(int)

    eorder = np.lexsort((w_e, sideB, core_e))
    srow_s = srow[eorder]
    dloc_s = dloc[eorder]
    core_s = core_e[eorder]
    sideB_s = sideB[eorder]
    w_s = w_e[eorder]
    cstarts = np.searchsorted(core_s, np.arange(NC + 1))

    idx_cores, oh_cores, oht_cores = [], [], []
    lo_cores, hi_cores, m_lo, m_hi = [], [], [], []
    for c in range(NC):
        lo_, hi_ = cstarts[c], cstarts[c + 1]
        sr_c = srow_s[lo_:hi_]
        dl_c = dloc_s[lo_:hi_]
        sd_c = sideB_s[lo_:hi_]
        ww_c = w_s[lo_:hi_]
        idx_blk = np.zeros((BA + BB, 128), np.int16)
        dl_blk = np.full((BA + BB, 128), -1, np.int64)
        bstart = np.searchsorted(sd_c, 1)
        for sideflag, nW, off, base, elo, ehi in (
            (False, nA, offA, 0, 0, bstart),
            (True, nB, offB, BA, bstart, len(sr_c)),
        ):
            sr = sr_c[elo:ehi] - (ASPLIT if sideflag else 0)
            dl = dl_c[elo:ehi]
            ww = ww_c[elo:ehi]
            starts = np.searchsorted(ww, np.arange(W + 1))
            for w in range(W):
                s0, s1 = starts[w], starts[w + 1]
                cnt = s1 - s0
                b0 = base + off[w]
                fa = idx_blk[b0 : b0 + nW[w]].reshape(-1)
                fa[:cnt] = sr[s0:s1]
                fd = dl_blk[b0 : b0 + nW[w]].reshape(-1)
                fd[:cnt] = dl[s0:s1]
        idx_cores.append(_pack_idx(idx_blk))
        ohb = np.zeros((BA + BB, 128, ND), np.uint8)
        bb, pp = np.nonzero(dl_blk >= 0)
        ohb[bb, pp, dl_blk[bb, pp]] = 1
        oh_cores.append(np.ascontiguousarray(
            ohb.transpose(1, 0, 2).reshape(128, -1).astype(F8NP)))
        oht_cores.append(np.ascontiguousarray(
            ohb.transpose(2, 0, 1).reshape(ND, -1).astype(F8NP)))

        # own-row gather indices (for layer-1 alpha_dst): lo/hi + masks.
        # own position j = w*ND + d; row depends on the chunk-major layout.
        jj = np.arange(cfg.pcn)
        wn = jj // ND
        hh = wn // CPW
        own = hh * HALF + c * HPC + (wn - hh * CPW) * ND + (jj % ND)
        is_lo = own < ASPLIT
        lo_idx = np.where(is_lo, own, 0).astype(np.int16)
        hi_idx = np.where(~is_lo, own - HISTART, 0).astype(np.int16)
        lo_cores.append(_pack_idx(lo_idx.reshape(-1, 128)))
        hi_cores.append(_pack_idx(hi_idx.reshape(-1, 128)))
        # mask per position, laid out [partition, sub, head]
        ml = is_lo.astype(np.float32)
        ml4 = np.repeat(ml[:, None], HEADS, 1).reshape(-1, 128, HEADS)
        ml4 = ml4.transpose(1, 0, 2).reshape(128, -1)
        m_lo.append(ml4.astype(BF))
        m_hi.append((1.0 - ml4).astype(BF))

    return dict(
        node_of_row=node_of_row,
        row_of_node=row_of_node,
        idx=idx_cores, oh=oh_cores, oht=oht_cores,
        idxlo=lo_cores, idxhi=hi_cores, mlo=m_lo, mhi=m_hi,
        BA=BA, BB=BB,
    )


def _blkdiag(a):
    out = np.zeros((HC, HEADS), np.float32)
    a = np.asarray(a, np.float32)
    for h in range(HEADS):
        out[h * CH : (h + 1) * CH, h] = a[h]
    return out


def build_inputs(cfg: Cfg, layout, x, W1, a_src1, a_dst1, b1, W2, a_src2,
                 a_dst2, b2, fc_w, fc_b):
    node_of_row = layout["node_of_row"]
    xs = np.zeros((cfg.rows, DIN), np.float32)
    valid = node_of_row >= 0
    xs[valid] = np.asarray(x, np.float32)[node_of_row[valid]]
    xbf = np.ascontiguousarray(xs.T).astype(BF)            # [128, rows]

    W1 = np.asarray(W1, np.float32)
    W2 = np.asarray(W2, np.float32)
    w1aug = np.concatenate(
        [W1, W1 @ _blkdiag(a_src1), W1 @ _blkdiag(a_dst1)], axis=1).astype(BF)
    w2full = np.concatenate(
        [W2, W2 @ _blkdiag(a_src2), W2 @ _blkdiag(a_dst2)], axis=1).astype(BF)
    w2aug = np.ascontiguousarray(w2full.reshape(2, 128, USED))

    def _bexp(b):
        # [128, (half, wi, d)] pre-expanded bias
        bc = np.asarray(b, np.float32).reshape(2, 128).T    # [128, half]
        return np.ascontiguousarray(
            np.broadcast_to(bc[:, :, None, None], (128, 2, 2, 64))
            .reshape(128, 256))

    b1c = _bexp(b1)
    b2c = _bexp(b2)
    fcw = np.ascontiguousarray(
        np.asarray(fc_w, np.float32).reshape(2, 128, 1).astype(BF))
    fcb = np.full((128, 1), np.float32(np.asarray(fc_b).reshape(-1)[0]))

    sel = np.zeros((2, 4, 128), np.float32)
    for half in range(2):
        for h in range(2):
            sel[half, 2 * half + h, h * CH : (h + 1) * CH] = 1.0
    sel = np.ascontiguousarray(sel)

    eye = np.zeros((2, 128, 64), np.float32)
    eye[0, np.arange(64), np.arange(64)] = 1.0
    eye[1, 64 + np.arange(64), np.arange(64)] = 1.0
    eye = eye.astype(BF)

    base = dict(xbf=xbf, w1aug=w1aug, w2aug=w2aug, b1c=b1c, b2c=b2c,
                fcw=fcw, fcb=fcb, sel=sel, eye=eye)
    in_maps = []
    assert cfg.cch == 1
    for c in range(cfg.nc):
        m = dict(base)
        m["idx"] = layout["idx"][c]
        m["oh"] = layout["oh"][c]
        m["oht"] = layout["oht"][c]
        # own-node inputs x (feature-major) for the direct adW1 projection
        m["xown"] = np.ascontiguousarray(
            xs[c * cfg.pcn : (c + 1) * cfg.pcn].T).astype(BF)
        in_maps.append(m)
    return in_maps


def build_program(cfg: Cfg, shared_out: bool = True):
    nc_b = bacc.Bacc(None, num_devices=cfg.nc)
    NC, ND, W, GCH = cfg.nc, cfg.ndst, cfg.wpc, cfg.gch
    nA, nB = cfg.nA, cfg.nB
    BA, BB = int(np.sum(nA)), int(np.sum(nB))
    NBLK = BA + BB
    ROWS = cfg.rows
    PCN = cfg.pcn
    NSUB = PCN // 128                                   # own-gather sub count
    offA = np.concatenate([[0], np.cumsum(nA)]).astype(int)
    offB = np.concatenate([[0], np.cumsum(nB)]).astype(int)

    xbfT = nc_b.dram_tensor("xbf", [DIN, ROWS], BF16, kind="ExternalInput")
    w1augT = nc_b.dram_tensor("w1aug", [DIN, USED], BF16, kind="ExternalInput")
    w2augT = nc_b.dram_tensor("w2aug", [2, 128, USED], BF16, kind="ExternalInput")
    b1cT = nc_b.dram_tensor("b1c", [128, 256], F32, kind="ExternalInput")
    b2cT = nc_b.dram_tensor("b2c", [128, 256], F32, kind="ExternalInput")
    fcwT = nc_b.dram_tensor("fcw", [2, 128, 1], BF16, kind="ExternalInput")
    fcbT = nc_b.dram_tensor("fcb", [128, 1], F32, kind="ExternalInput")
    selT = nc_b.dram_tensor("sel", [2, 4, 128], F32, kind="ExternalInput")
    idxT = nc_b.dram_tensor("idx", [128, NBLK * 8], I16, kind="ExternalInput")
    ohT = nc_b.dram_tensor("oh", [128, NBLK * ND], F8, kind="ExternalInput")
    ohtT = nc_b.dram_tensor("oht", [ND, NBLK * 128], F8, kind="ExternalInput")
    xownT = nc_b.dram_tensor("xown", [DIN, PCN], BF16, kind="ExternalInput")
    eyeT = nc_b.dram_tensor("eye", [2, 128, 64], BF16, kind="ExternalInput")
    yT = nc_b.dram_tensor("y", [128, W // 2], F32, kind="ExternalOutput")

    HPCg = PCN // cfg.cch
    HALFg = ROWS // cfg.cch
    table1 = nc_b.dram_tensor("table1", [ROWS, EW], BF16)
    tab2own_t = [
        nc_b.dram_tensor(f"tab2own{k}", [HPCg, EW], BF16)
        for k in range(cfg.cch)
    ]
    table2_t = [
        nc_b.dram_tensor(
            f"table2_{k}", [HALFg, EW], BF16,
            addr_space="Shared" if shared_out else "Local")
        for k in range(cfg.cch)
    ]

    # chunk plan over the A-stream then B-stream of blocks
    chunks = []
    for base, nb in ((0, BA), (BA, BB)):
        b = 0
        while b < nb:
            wdt = min(GCH, nb - b)
            chunks.append((base + b, wdt, base == BA))
            b += wdt
    chunk_of_blk = {}
    for ci, (b0, cw, _) in enumerate(chunks):
        for k in range(cw):
            chunk_of_blk[b0 + k] = (ci, k)

    win_of_blk = np.zeros(NBLK, int)
    for w in range(W):
        win_of_blk[offA[w] : offA[w + 1]] = w
        win_of_blk[BA + offB[w] : BA + offB[w + 1]] = w

    CPW = W // cfg.cch

    import contextlib

    with tile.TileContext(nc_b) as tc:
        ctx = [
            tc.tile_pool(name="cst", bufs=1),
            tc.tile_pool(name="dx", bufs=POOLS["dx"]),
            tc.tile_pool(name="dh", bufs=POOLS["dh"]),
            tc.tile_pool(name="ixp", bufs=3),
            tc.tile_pool(name="gtA", bufs=TUNE["gt"]),
            tc.tile_pool(name="gtB", bufs=TUNE["gt"]),
            tc.tile_pool(name="er", bufs=TUNE["er"]),
            tc.tile_pool(name="e4A", bufs=TUNE["e4"]),
            tc.tile_pool(name="e4B", bufs=TUNE["e4"]),
            tc.tile_pool(name="og", bufs=2),
            tc.tile_pool(name="ohA", bufs=TUNE["oh"]),
            tc.tile_pool(name="ohB", bufs=TUNE["oh"]),
            tc.tile_pool(name="otA", bufs=TUNE["ot"]),
            tc.tile_pool(name="otB", bufs=TUNE["ot"]),
            tc.tile_pool(name="sm", bufs=TUNE["sm"]),
            tc.tile_pool(name="fin", bufs=TUNE["fin"]),
            tc.tile_pool(name="sp", bufs=1),
            tc.tile_pool(name="pa", bufs=POOLS["pa"], space="PSUM"),
            tc.tile_pool(name="pg", bufs=POOLS["pg"], space="PSUM"),
            tc.tile_pool(name="pp", bufs=POOLS["pp"], space="PSUM"),
        ]
        with contextlib.ExitStack() as st:
            (cst, dx, dh, ixp, gtA, gtB, er, e4A, e4B, og,
             ohA, ohB, otA, otB, sm, fin, sp,
             pa, pg, pp) = [st.enter_context(m) for m in ctx]

            # ---- constants ----
            w1s = cst.tile([128, USED], BF16)
            nc_b.sync.dma_start(out=w1s[:], in_=w1augT[:, :])
            w2s = cst.tile([128, 2, USED], BF16)
            nc_b.sync.dma_start(out=w2s[:, 0, :], in_=w2augT[0, :, :])
            nc_b.sync.dma_start(out=w2s[:, 1, :], in_=w2augT[1, :, :])
            b1s = cst.tile([128, 256], F32)
            nc_b.sync.dma_start(out=b1s[:], in_=b1cT[:, :])
            b2s = cst.tile([128, 256], F32)
            nc_b.sync.dma_start(out=b2s[:], in_=b2cT[:, :])
            fcws = cst.tile([128, 2, 1], BF16)
            nc_b.sync.dma_start(out=fcws[:, 0, :], in_=fcwT[0, :, :])
            nc_b.sync.dma_start(out=fcws[:, 1, :], in_=fcwT[1, :, :])
            fcbs = cst.tile([128, 1], F32)
            nc_b.sync.dma_start(out=fcbs[:], in_=fcbT[:, :])
            sels = cst.tile([4, 2, 128], F32)
            nc_b.sync.dma_start(out=sels[:, 0, :], in_=selT[0, :, :])
            nc_b.sync.dma_start(out=sels[:, 1, :], in_=selT[1, :, :])
            ixs = cst.tile([128, NBLK * 8], I16)
            nc_b.sync.dma_start(out=ixs[:], in_=idxT[:, :])
            xos = cst.tile([128, PCN], BF16)
            nc_b.sync.dma_start(out=xos[:], in_=xownT[:, :])
            eyes = cst.tile([128, 2, 64], BF16)
            nc_b.sync.dma_start(out=eyes[:, 0, :], in_=eyeT[0, :, :])
            nc_b.sync.dma_start(out=eyes[:, 1, :], in_=eyeT[1, :, :])
            adW1 = cst.tile([ND, W, HEADS], BF16)
            adW2 = cst.tile([ND, W, HEADS], BF16)
            zAll = cst.tile([128, W // 2], F32)
            negones = cst.tile([128, 1], F32)
            nc_b.vector.memset(negones[:], -1.0)
            adraw2 = cst.tile([128, W // 2, HEADS], BF16)

            # ---- dense phase (replicated): table1 rows = [x @ W1aug] ----
            NT8 = ROWS // 1024
            for t8 in range(NT8):
                xin = dx.tile([128, 8, 128], BF16, tag="xin")
                nc_b.sync.dma_start(
                    out=xin[:], in_=xbfT[:, t8 * 1024 : (t8 + 1) * 1024])
                hb = dh.tile([128, 8, USED], BF16, tag="hb")
                for i in range(8):
                    pht = pp.tile([128, USED], F32, space="PSUM", tag="pp")
                    nc_b.tensor.matmul(
                        out=pht[:], lhsT=xin[:, i, :], rhs=w1s[:],
                        start=True, stop=True)
                    if i % 2 == 0:
                        nc_b.vector.tensor_copy(out=hb[:, i, :], in_=pht[:])
                    else:
                        nc_b.scalar.copy(out=hb[:, i, :], in_=pht[:])
                nc_b.sync.dma_start(
                    out=bass.AP(
                        tensor=table1[:, :].tensor,
                        offset=t8 * 1024 * EW,
                        ap=[[EW, 128], [EW * 128, 8], [1, USED]],
                    ),
                    in_=bass.AP(
                        tensor=hb[:].tensor, offset=hb[:].offset,
                        ap=[hb[:].ap[0], [USED, 8], [1, USED]],
                    ),
                )

            # ---- adW1 directly from x_own @ (W1 a_dst1) ----
            # ad1[node] is a projection of the INPUT x, so it needs neither
            # table1 nor any own-row gather: 49 tiny PE matmuls against the
            # ad columns of w1s.  This unblocks the Pool queue -- layer-1
            # edge gathers now start as soon as table1's A-range is written.
            adraw = cst.tile([128, NSUB, HEADS], BF16)
            adO = pa.tile([128, 196], F32, space="PSUM", tag="ad")
            for p in range(NSUB):
                nc_b.tensor.matmul(
                    out=adO[:, p * HEADS : (p + 1) * HEADS],
                    lhsT=xos[:, p * 128 : (p + 1) * 128],
                    rhs=w1s[:, AS_OFF + HEADS : USED],
                    start=True, stop=True)
            nc_b.vector.tensor_copy(
                out=bass.AP(
                    tensor=adraw[:].tensor, offset=adraw[:].offset,
                    ap=[adraw[:].ap[0], [1, NSUB * HEADS]],
                ),
                in_=adO[:, 0 : NSUB * HEADS])
            for par in range(2):
                adT = pa.tile([128, 196], F32, space="PSUM", tag="ad")
                adPP = adT[0:ND, 0 : NSUB * HEADS]
                nc_b.tensor.matmul(
                    out=adPP, lhsT=eyes[:, par, :], rhs=adraw[:],
                    start=True, stop=True)
                nc_b.vector.tensor_copy(
                    out=bass.AP(
                        tensor=adW1[:].tensor,
                        offset=adW1[:].offset + par * HEADS,
                        ap=[adW1[:].ap[0], [2 * HEADS, NSUB], [1, HEADS]],
                    ),
                    in_=adPP)

            def cc_chunk(k):
                nc_b.gpsimd.collective_compute(
                    "AllGather",
                    OP.bypass,
                    replica_groups=[list(range(NC))],
                    ins=[tab2own_t[k][:, :].opt()],
                    outs=[table2_t[k][:, :].opt()],
                )

            def mk_state(layer):
                st = dict(
                    emitted=[False] * len(chunks),
                    preloaded=[None] * len(chunks),
                    preadp=[None] * len(chunks),
                    ctiles=[None] * len(chunks),
                )

                def emit_loads(ci):
                    # SP-only part: prefetchable many chunks ahead
                    if st["preloaded"][ci] is not None:
                        return st["preloaded"][ci]
                    b0, cw, is_b = chunks[ci]
                    oh_p = ohB if is_b else ohA
                    ot_p = otB if is_b else otA
                    oh_t = oh_p.tile([128, GCH * ND], F8, tag="oh")
                    nc_b.sync.dma_start(
                        out=oh_t[:, 0 : cw * ND],
                        in_=ohT[:, b0 * ND : (b0 + cw) * ND])
                    oht_t = ot_p.tile([ND, GCH * 128], F8, tag="oht")
                    nc_b.sync.dma_start(
                        out=oht_t[:, 0 : cw * 128],
                        in_=ohtT[:, b0 * 128 : (b0 + cw) * 128])
                    st["preloaded"][ci] = (oh_t, oht_t)
                    return st["preloaded"][ci]

                def emit_adp(ci):
                    # PE part, done near emit_chunk time (bounded PSUM use)
                    if st["preadp"][ci] is not None:
                        return st["preadp"][ci]
                    b0, cw, is_b = chunks[ci]
                    adW = adW1 if layer == 1 else adW2
                    oh_t, oht_t = emit_loads(ci)
                    adT = pa.tile([128, 196], F32, space="PSUM", tag="ad")
                    adP = adT[:, 0 : GCH * HEADS]
                    for k in range(cw):
                        wv = int(win_of_blk[b0 + k])
                        nc_b.tensor.matmul(
                            out=adP[:, k * HEADS : (k + 1) * HEADS],
                            lhsT=oht_t[:, k * 128 : (k + 1) * 128],
                            rhs=adW[:, wv, :],
                            start=True, stop=True)
                    st["preadp"][ci] = (oh_t, adP)
                    return st["preadp"][ci]

                st["emit_loads"] = emit_loads
                st["emit_adp"] = emit_adp
                return st

            # chunk index ranges per side (for load lookahead)
            nA_chunks = len([c for c in chunks if not c[2]])

            def edge_layer(layer, st, pre_cc=None):
                emitted = st["emitted"]
                ctiles = st["ctiles"]
                emit_loads = st["emit_loads"]
                emit_adp = st["emit_adp"]

                def emit_chunk(ci):
                    b0, cw, is_b = chunks[ci]
                    gt_p = gtB if is_b else gtA
                    e4_p = e4B if is_b else e4A
                    # prefetch loads for upcoming chunks of this side
                    lo, hi = ((nA_chunks, len(chunks)) if is_b
                              else (0, nA_chunks))
                    for cj in range(ci, min(ci + LOOKAHEAD, hi)):
                        emit_loads(cj)
                    oh_t, adP = emit_adp(ci)
                    for cj in range(ci + 1, min(ci + ADLA + 1, hi)):
                        emit_adp(cj)
                    g = gt_p.tile([128, GCH, EW], BF16, tag="g")
                    if layer == 1:
                        tab = table1
                    elif cfg.cch == 1:
                        tab = table2_t[0]
                    else:
                        assert ASPLIT == HALFg
                        tab = table2_t[1 if is_b else 0]
                    if layer == 1 or cfg.cch == 1:
                        in_ap = (tab[ASPLIT:ROWS, :] if is_b
                                 else tab[0:ASPLIT, :])
                    else:
                        in_ap = tab[:, :]
                    for q0 in range(0, cw, GQ):
                        qw = min(GQ, cw - q0)
                        nc_b.gpsimd.dma_gather(
                            out_ap=bass.AP(
                                tensor=g[:].tensor,
                                offset=g[:].offset + q0 * EW,
                                ap=[g[:].ap[0], [EW, qw], [1, EW]],
                            ),
                            in_ap=in_ap,
                            idxs_ap=ixs[:, (b0 + q0) * 8 : (b0 + q0 + qw) * 8],
                            num_idxs=128 * qw,
                            num_idxs_reg=128 * qw,
                            elem_size=EW,
                        )
                    e0 = sm.tile([128, GCH * HEADS], BF16, tag="e0")
                    nc_b.vector.tensor_tensor(
                        out=e0[:, 0 : cw * HEADS],
                        in0=bass.AP(
                            tensor=g[:].tensor, offset=g[:].offset + AS_OFF,
                            ap=[g[:].ap[0], [EW, cw], [1, HEADS]],
                        ),
                        in1=adP[:, 0 : cw * HEADS],
                        op=OP.add)
                    e2 = sm.tile([128, GCH * HEADS], BF16, tag="e2")
                    nc_b.vector.scalar_tensor_tensor(
                        out=e2[:, 0 : cw * HEADS], in0=e0[:, 0 : cw * HEADS],
                        scalar=NEG, in1=e0[:, 0 : cw * HEADS],
                        op0=OP.mult, op1=OP.max)
                    # exp broadcast split: ACT fills cols [0:ES), DVE
                    # replicates the rest with a 4x-mode copy
                    ert = er.tile([128, GCH, HEADS, CH], BF16, tag="er")
                    nc_b.scalar.activation(
                        out=bass.AP(
                            tensor=ert[:].tensor, offset=ert[:].offset,
                            ap=[ert[:].ap[0], [CH, cw * HEADS], [1, ES]],
                        ),
                        in_=bass.AP(
                            tensor=e2[:].tensor, offset=e2[:].offset,
                            ap=[e2[:].ap[0], [1, cw * HEADS], [0, ES]],
                        ),
                        func=ACT.Exp)
                    if ES < CH:
                        nc_b.vector.tensor_copy(
                            out=bass.AP(
                                tensor=ert[:].tensor,
                                offset=ert[:].offset + ES,
                                ap=[ert[:].ap[0], [CH, cw * HEADS],
                                    [1, CH - ES]],
                            ),
                            in_=bass.AP(
                                tensor=ert[:].tensor, offset=ert[:].offset,
                                ap=[ert[:].ap[0], [CH, cw * HEADS],
                                    [1, CH - ES]],
                            ))
                    # scale h in place inside the gather tile (saves SBUF)
                    gpt = g
                    nc_b.vector.tensor_tensor(
                        out=bass.AP(
                            tensor=g[:].tensor, offset=g[:].offset,
                            ap=[g[:].ap[0], [EW, cw], [1, HC]],
                        ),
                        in0=bass.AP(
                            tensor=g[:].tensor, offset=g[:].offset,
                            ap=[g[:].ap[0], [EW, cw], [1, HC]],
                        ),
                        in1=ert[:, 0:cw, :, :],
                        op=OP.mult)
                    exp4 = e4_p.tile([128, GCH, HEADS], BF16, tag="e4")
                    nc_b.vector.tensor_copy(
                        out=exp4[:, 0:cw, :],
                        in_=bass.AP(
                            tensor=ert[:].tensor, offset=ert[:].offset,
                            ap=[ert[:].ap[0], [HEADS * CH, cw], [CH, HEADS]],
                        ))
                    emitted[ci] = True
                    ctiles[ci] = (gpt, exp4, oh_t)

                # windows processed in PAIRS: one PSUM tile + one finalize
                # chain per two windows (layout [128, wi, {seg0,seg1,den}, ND])
                for w in range(0, W, 2):
                    # PSUM layout [128, {seg_half0, seg_half1, den}, wi, ND]
                    segF = pg.tile([128, 3, 2, ND], F32, space="PSUM",
                                   tag="seg")
                    first_mm = True
                    lasts = []
                    for wi in range(2):
                        bl = list(range(offA[w + wi], offA[w + wi + 1])) + \
                             list(range(BA + offB[w + wi],
                                        BA + offB[w + wi + 1]))
                        lasts.append(bl[-1])
                    for wi in range(2):
                        bl = list(range(offA[w + wi], offA[w + wi + 1])) + \
                             list(range(BA + offB[w + wi],
                                        BA + offB[w + wi + 1]))
                        for b in bl:
                            ci, k = chunk_of_blk[b]
                            if not emitted[ci]:
                                emit_chunk(ci)
                            gpt, exp4, oh_t = ctiles[ci]
                            for half in range(2):
                                nc_b.tensor.matmul(
                                    out=segF[:, half, wi, :],
                                    lhsT=bass.AP(
                                        tensor=gpt[:].tensor,
                                        offset=(gpt[:].offset + k * EW
                                                + half * 128),
                                        ap=[gpt[:].ap[0], [1, 128]],
                                    ),
                                    rhs=oh_t[:, k * ND : (k + 1) * ND],
                                    start=first_mm, stop=False)
                                first_mm = False
                            nc_b.tensor.matmul(
                                out=segF[0:4, 2, wi, :],
                                lhsT=exp4[:, k, :],
                                rhs=oh_t[:, k * ND : (k + 1) * ND],
                                start=False,
                                stop=(wi == 1 and b == lasts[1]))

                    # ---- finalize window pair (w, w+1) ----
                    den_s = fin.tile([4, 2, ND], F32, tag="dens")
                    nc_b.vector.reciprocal(
                        out=den_s[:], in_=segF[0:4, 2, :, :])
                    rdT = pa.tile([128, 2, 2, ND], F32, space="PSUM",
                                  tag="ad")
                    for half in range(2):
                        nc_b.tensor.matmul(
                            out=rdT[:, half, :, :],
                            lhsT=sels[:, half, :],
                            rhs=den_s[:],
                            start=True, stop=True)
                    # flat [128, (wi, half, d)] layout for finalize tensors
                    rdenS = fin.tile([128, 2 * 2 * ND], F32, tag="rdenS")
                    nc_b.scalar.copy(
                        out=rdenS[:],
                        in_=bass.AP(
                            tensor=rdT[:].tensor, offset=rdT[:].offset,
                            ap=[rdT[:].ap[0], [1, 2 * 2 * ND]],
                        ))
                    nrm = fin.tile([128, 2 * 2 * ND], F32, tag="nrm")
                    nc_b.vector.tensor_tensor(
                        out=nrm[:],
                        in0=bass.AP(
                            tensor=segF[:].tensor, offset=segF[:].offset,
                            ap=[segF[:].ap[0], [1, 2 * 2 * ND]],
                        ),
                        in1=rdenS[:], op=OP.mult)
                    bsel = b1s if layer == 1 else b2s
                    xb = fin.tile([128, 2 * 2 * ND], F32, tag="xb")
                    nc_b.vector.tensor_tensor(
                        out=xb[:], in0=nrm[:], in1=bsel[:], op=OP.add)
                    em = fin.tile([128, 2 * 2 * ND], F32, tag="em")
                    nc_b.scalar.activation(out=em[:], in_=xb[:], func=ACT.Exp)
                    rl = fin.tile([128, 2 * 2 * ND], F32, tag="rl")
                    nc_b.vector.scalar_tensor_tensor(
                        out=rl[:], in0=xb[:], scalar=0.0,
                        in1=bass.AP(
                            tensor=negones[:].tensor,
                            offset=negones[:].offset,
                            ap=[negones[:].ap[0], [0, 2 * 2 * ND]],
                        ),
                        op0=OP.max, op1=OP.add)
                    eluT = fin.tile([128, 2, 2, ND], BF16, tag="eluT")
                    nc_b.vector.scalar_tensor_tensor(
                        out=bass.AP(
                            tensor=eluT[:].tensor, offset=eluT[:].offset,
                            ap=[eluT[:].ap[0], [1, 2 * 2 * ND]],
                        ),
                        in0=em[:], scalar=1.0, in1=rl[:],
                        op0=OP.min, op1=OP.add)
                    ppt = pp.tile([128, USED], F32, space="PSUM", tag="pp")
                    if layer == 1:
                        h2P = ppt[:, :]
                        for half in range(2):
                            nc_b.tensor.matmul(
                                out=h2P,
                                lhsT=eluT[:, half, :, :],
                                rhs=w2s[:, half, :],
                                start=(half == 0), stop=(half == 1))
                        h2b = fin.tile([128, EW], BF16, tag="h2b")
                        nc_b.scalar.copy(out=h2b[:, 0:USED], in_=h2P)
                        # stash this pair's alpha_dst rows ([wi,d] partitions)
                        nc_b.sync.dma_start(
                            out=adraw2[:, w // 2, :],
                            in_=h2b[:, AD_OFF:USED])
                        nc_b.sync.dma_start(
                            out=tab2own_t[0][w * ND : (w + 2) * ND, :],
                            in_=h2b[:])
                        if w + 2 == W:
                            for par in range(2):
                                adT2 = pa.tile([128, 196], F32,
                                               space="PSUM", tag="ad")
                                adPP2 = adT2[0:ND, 0 : (W // 2) * HEADS]
                                nc_b.tensor.matmul(
                                    out=adPP2, lhsT=eyes[:, par, :],
                                    rhs=adraw2[:],
                                    start=True, stop=True)
                                nc_b.vector.tensor_copy(
                                    out=bass.AP(
                                        tensor=adW2[:].tensor,
                                        offset=(adW2[:].offset
                                                + par * HEADS),
                                        ap=[adW2[:].ap[0],
                                            [2 * HEADS, W // 2],
                                            [1, HEADS]],
                                    ),
                                    in_=adPP2)
                            if pre_cc is not None:
                                pre_cc()
                            cc_chunk(0)
                    else:
                        zP = ppt[:, 0:1]
                        for half in range(2):
                            nc_b.tensor.matmul(
                                out=zP,
                                lhsT=eluT[:, half, :, :],
                                rhs=fcws[:, half, :],
                                start=(half == 0), stop=(half == 1))
                        nc_b.vector.tensor_copy(
                            out=zAll[:, w // 2 : w // 2 + 1], in_=zP)

            marks = {}
            nc_b._phase_marks = marks
            st1 = mk_state(1)
            st2 = mk_state(2)
            firstB = len([c for c in chunks if not c[2]])

            def prefetch2():
                # stream layer-2 one-hot loads under the collective
                for ci in range(min(CCPRE, firstB)):
                    st2["emit_loads"](ci)
                for ci in range(firstB, min(firstB + CCPRE, len(chunks))):
                    st2["emit_loads"](ci)

            marks["setup_end"] = len(nc_b.inst_map)
            edge_layer(1, st1, pre_cc=prefetch2)
            marks["e1_end"] = len(nc_b.inst_map)
            edge_layer(2, st2)
            marks["e2_end"] = len(nc_b.inst_map)

            ysig = cst.tile([128, W // 2], F32)
            nc_b.scalar.activation(
                out=ysig[:], in_=zAll[:], func=ACT.Sigmoid,
                bias=fcbs[:, 0:1], scale=1.0)
            nc_b.sync.dma_start(out=yT[:, :], in_=ysig[:])

    nc_b.finalize()
    return nc_b


def assemble_output(cfg: Cfg, layout, results):
    node_of_row = layout["node_of_row"]
    yfull = np.zeros((cfg.n_real, 1), np.float32)
    CPW = cfg.wpc // cfg.cch
    HPC = cfg.pcn // cfg.cch
    HALF = cfg.rows // cfg.cch
    for c in range(cfg.nc):
        yc = np.asarray(results[c]["y"])               # [128, 49]
        q, p = np.meshgrid(np.arange(128), np.arange(cfg.wpc // 2),
                           indexing="ij")
        w = 2 * p + q // cfg.ndst
        d = q % cfg.ndst
        hh = w // CPW
        rows = hh * HALF + c * HPC + (w - hh * CPW) * cfg.ndst + d
        nodes = node_of_row[rows.reshape(-1)]
        ok = nodes >= 0
        yfull[nodes[ok], 0] = yc.reshape(-1)[ok]
    return yfull


def _absorb_device_wedge():
    """Run a trivial 8-core kernel; a crashed prior session can leave the
    NeuronCores in NRT_EXEC_UNIT_UNRECOVERABLE state, which a fresh trivial
    execution clears."""
    try:
        from concourse.bass_utils import run_bass_kernel_spmd

        nc_t = bacc.Bacc(None, num_devices=8)
        a = nc_t.dram_tensor("a", [128, 128], F32, kind="ExternalInput")
        o = nc_t.dram_tensor("o", [128, 128], F32, kind="ExternalOutput")
        with tile.TileContext(nc_t) as tc:
            with tc.tile_pool(name="sb", bufs=1) as sb:
                t = sb.tile([128, 128], F32)
                nc_t.sync.dma_start(out=t[:], in_=a[:, :])
                nc_t.sync.dma_start(out=o[:, :], in_=t[:])
        nc_t.finalize()
        run_bass_kernel_spmd(
            nc_t, [{"a": np.zeros((128, 128), np.float32)}] * 8,
            core_ids=list(range(8)),
        )
    except Exception:
        pass


def kernel(**inputs):
    from concourse.bass_utils import run_bass_kernel_spmd

    cfg = Cfg()
    layout = build_layout(inputs["edge_index"], cfg)
    in_maps = build_inputs(
        cfg, layout,
        inputs["x"], inputs["W1"], inputs["a_src1"], inputs["a_dst1"],
        inputs["b1"], inputs["W2"], inputs["a_src2"], inputs["a_dst2"],
        inputs["b2"], inputs["fc_w"], inputs["fc_b"],
    )
    nc_b = build_program(cfg, shared_out=True)
    last_err = None
    for attempt in range(3):
        try:
            res = run_bass_kernel_spmd(
                nc_b, in_maps, core_ids=list(range(cfg.nc)))
            return assemble_output(cfg, layout, res.results)
        except Exception as e:  # wedged device from a prior crashed session
            last_err = e
            _absorb_device_wedge()
    raise last_err


if __name__ == "__main__":
    pass



# revision 105
# speedup vs baseline: 1.0144x; 1.0068x over previous
"""Two-layer GAT (PyG GATConv-style) on 8 Trainium2 NeuronCores via Bass/Tile.

Edges-on-partitions design (v3, pair-window pipeline):
  - Nodes are degree-stratified into 98 strata of 512; each stratum contributes
    64 nodes to every core (snake order), giving core-major table rows
    row = core*6272 + window*64 + slot.  Window = 64 destination nodes.
  - A bf16 feature table holds rows [h(256) | alpha_src(4) | alpha_dst(4) | pad]
    with 768 B stride.  Layer-1 table is computed replicated (dense bf16
    matmuls); layer-2 table rows are produced per window pair by a fused dense
    matmul and AllGathered across the 8 cores (the AllGather hard-blocks the
    Pool queue in the cost model, so a single cch=1 collective is optimal).
  - Each core processes its ~106k incoming edges as 128-edge blocks (sorted by
    window, split into A/B streams at ASPLIT=25088 so gather indices fit
    int16).  SWDGE dma_gather fetches h[src] rows for gch=7 blocks per
    instruction; the steady state is gather-stream bound (back-to-back).
  - Attention: alpha_dst per edge via one-hot^T matmul (one-hots stored in
    fp8e4 -- PE matmuls take mixed bf16/fp8 operands); e = lrelu(as+ad) on
    DVE; exp broadcast to [4,64] on Activation; G' = G * exp multiplied IN
    PLACE in the gather tile (DVE 2x); segment sum + softmax denominator via
    PE matmuls accumulated per WINDOW PAIR in one PSUM tile
    [128, {seg0,seg1,den}, wi, 64].  No segment-max needed.
  - Pair finalize (flat [128,(half,wi,d)] layout, >=3-dim-AP verifier-safe):
    1/den via reciprocal + sel-matmul broadcast + one ACT copy; bias add from
    a pre-expanded [128,256] bias tile; ELU as min(exp(xb),1) + (max(xb,0)-1);
    h2/fc matmuls cover both windows in one 128-partition lhsT.  alpha_dst
    for layer 2 is stashed per pair via SBUF->SBUF DMA of the h2b ad columns
    and assembled with eyes-matmuls at end of layer 1; for layer 1 it comes
    from own-row gathers that fetch only the 256 B row tail (elem_step=EW).
  - Load pipeline: one-hot loads prefetched LOOKAHEAD chunks ahead, adP
    matmuls ADLA chunks ahead, CCPRE layer-2 load chunks streamed under the
    collective.  One Sigmoid at the very end ([128, 49] pair-major output).
"""

import sys

sys.path.insert(0, "/opt/trn_rl_repo")

from dataclasses import dataclass, field

import numpy as np
import ml_dtypes

BF = ml_dtypes.bfloat16
F8NP = ml_dtypes.float8_e4m3fn

import concourse.bass as bass
import concourse.bacc as bacc
import concourse.tile as tile
from concourse import mybir

F32 = mybir.dt.float32
BF16 = mybir.dt.bfloat16
F8 = mybir.dt.float8e4
I16 = mybir.dt.int16
OP = mybir.AluOpType
ACT = mybir.ActivationFunctionType

HEADS = 4
CH = 64
HC = 256
DIN = 128
NEG = 0.2
EW = 384                 # table row width in bf16 elems (768 B)
USED = 264               # used columns: h(256) + as(4) + ad(4)
AS_OFF = 256
AD_OFF = 260
ASPLIT = 25088           # first B-range table row (= chunk boundary for cch=2)
HISTART = 50176 - 32768  # = 17408, start row of the hi own-gather range
TUNE = dict(gt=7, gp=2, e4=6, oh=10, ot=10, er=5, fin=3, sm=8)
GQ = 16                  # blocks per dma_gather instruction
LOOKAHEAD = 3            # one-hot load prefetch depth (chunks, per side)
CCPRE = 12               # chunks of L2 loads streamed under the collective
ADLA = 2                 # adP matmul lookahead (chunks)
ES = 64                  # exp-broadcast columns computed on ACT (rest: DVE)
POOLS = dict(dx=4, dh=4, pa=3, pg=2, pp=3)


@dataclass
class Cfg:
    n_real: int = 50000
    nc: int = 8
    ndst: int = 64               # dst nodes per window
    wpc: int = 98                # windows per core
    gch: int = 7                 # blocks per chunk tile
    cch: int = 1                 # collective chunks for table2
    nA: list = field(default_factory=list)   # per-window A-block counts
    nB: list = field(default_factory=list)   # per-window B-block counts
    # kept for test.py compatibility (prints sumK)
    ka: list = field(default_factory=list)
    kb: list = field(default_factory=list)

    @property
    def pcn(self):
        return self.wpc * self.ndst          # nodes per core (6272)

    @property
    def rows(self):
        return self.nc * self.pcn            # table rows (50176)


def _pack_idx(blk):
    """blk: [nblk, 128] int16 -> wrapped-16 layout [128, nblk*8]."""
    nblk = blk.shape[0]
    pk = blk.reshape(nblk, 8, 16).transpose(2, 0, 1).reshape(16, nblk * 8)
    return np.ascontiguousarray(np.tile(pk, (8, 1)).astype(np.int16))


def build_layout(edge_index, cfg: Cfg):
    n = cfg.n_real
    NC, ND, W = cfg.nc, cfg.ndst, cfg.wpc
    src = np.asarray(edge_index[0], dtype=np.int64)
    dst = np.asarray(edge_index[1], dtype=np.int64)
    src = np.concatenate([src, np.arange(n, dtype=np.int64)])
    dst = np.concatenate([dst, np.arange(n, dtype=np.int64)])
    deg = np.bincount(dst, minlength=n)

    order = np.argsort(-deg, kind="stable")          # degree-descending
    node_of_row = np.full(cfg.rows, -1, np.int64)
    row_of_node = np.full(n, -1, np.int64)
    j = np.arange(512)
    r8 = j // 8
    c8 = j % 8
    core_j = np.where(r8 % 2 == 0, c8, 7 - c8)
    slot_j = r8
    CPW = W // cfg.cch                      # windows per collective chunk
    HPC = cfg.pcn // cfg.cch                # rows per core per chunk (3136)
    HALF = cfg.rows // cfg.cch              # rows per chunk (25088)
    for s in range(W):
        nodes = order[s * 512 : (s + 1) * 512]
        hh = s // CPW
        rows = (hh * HALF + core_j[: len(nodes)] * HPC
                + (s - hh * CPW) * ND + slot_j[: len(nodes)])
        node_of_row[rows] = nodes
        row_of_node[nodes] = rows

    drow = row_of_node[dst]
    hh_e = drow // HALF
    rem = drow % HALF
    core_e = rem // HPC
    loc2 = rem % HPC
    w_e = hh_e * CPW + loc2 // ND
    dloc = loc2 % ND
    srow = row_of_node[src]
    sideB = srow >= ASPLIT

    cntA = np.zeros((NC, W), np.int64)
    cntB = np.zeros((NC, W), np.int64)
    np.add.at(cntA, (core_e[~sideB], w_e[~sideB]), 1)
    np.add.at(cntB, (core_e[sideB], w_e[sideB]), 1)
    nA = np.maximum(1, np.ceil(cntA.max(axis=0) / 128).astype(np.int64))
    nB = np.maximum(1, np.ceil(cntB.max(axis=0) / 128).astype(np.int64))
    cfg.nA = nA.tolist()
    cfg.nB = nB.tolist()
    cfg.ka = nA.tolist()
    cfg.kb = nB.tolist()
    BA, BB = int(nA.sum()), int(nB.sum())
    offA = np.concatenate([[0], np.cumsum(nA)]).astype(int)
    offB = np.concatenate([[0], np.cumsum(nB)]).astype(int)

    eorder = np.lexsort((w_e, sideB, core_e))
    srow_s = srow[eorder]
    dloc_s = dloc[eorder]
    core_s = core_e[eorder]
    sideB_s = sideB[eorder]
    w_s = w_e[eorder]
    cstarts = np.searchsorted(core_s, np.arange(NC + 1))

    idx_cores, oh_cores, oht_cores = [], [], []
    lo_cores, hi_cores, m_lo, m_hi = [], [], [], []
    for c in range(NC):
        lo_, hi_ = cstarts[c], cstarts[c + 1]
        sr_c = srow_s[lo_:hi_]
        dl_c = dloc_s[lo_:hi_]
        sd_c = sideB_s[lo_:hi_]
        ww_c = w_s[lo_:hi_]
        idx_blk = np.zeros((BA + BB, 128), np.int16)
        dl_blk = np.full((BA + BB, 128), -1, np.int64)
        bstart = np.searchsorted(sd_c, 1)
        for sideflag, nW, off, base, elo, ehi in (
            (False, nA, offA, 0, 0, bstart),
            (True, nB, offB, BA, bstart, len(sr_c)),
        ):
            sr = sr_c[elo:ehi] - (ASPLIT if sideflag else 0)
            dl = dl_c[elo:ehi]
            ww = ww_c[elo:ehi]
            starts = np.searchsorted(ww, np.arange(W + 1))
            for w in range(W):
                s0, s1 = starts[w], starts[w + 1]
                cnt = s1 - s0
                b0 = base + off[w]
                fa = idx_blk[b0 : b0 + nW[w]].reshape(-1)
                fa[:cnt] = sr[s0:s1]
                fd = dl_blk[b0 : b0 + nW[w]].reshape(-1)
                fd[:cnt] = dl[s0:s1]
        idx_cores.append(_pack_idx(idx_blk))
        ohb = np.zeros((BA + BB, 128, ND), np.uint8)
        bb, pp = np.nonzero(dl_blk >= 0)
        ohb[bb, pp, dl_blk[bb, pp]] = 1
        oh_cores.append(np.ascontiguousarray(
            ohb.transpose(1, 0, 2).reshape(128, -1).astype(F8NP)))
        oht_cores.append(np.ascontiguousarray(
            ohb.transpose(2, 0, 1).reshape(ND, -1).astype(F8NP)))

        # own-row gather indices (for layer-1 alpha_dst): lo/hi + masks.
        # own position j = w*ND + d; row depends on the chunk-major layout.
        jj = np.arange(cfg.pcn)
        wn = jj // ND
        hh = wn // CPW
        own = hh * HALF + c * HPC + (wn - hh * CPW) * ND + (jj % ND)
        is_lo = own < ASPLIT
        lo_idx = np.where(is_lo, own, 0).astype(np.int16)
        hi_idx = np.where(~is_lo, own - HISTART, 0).astype(np.int16)
        lo_cores.append(_pack_idx(lo_idx.reshape(-1, 128)))
        hi_cores.append(_pack_idx(hi_idx.reshape(-1, 128)))
        # mask per position, laid out [partition, sub, head]
        ml = is_lo.astype(np.float32)
        ml4 = np.repeat(ml[:, None], HEADS, 1).reshape(-1, 128, HEADS)
        ml4 = ml4.transpose(1, 0, 2).reshape(128, -1)
        m_lo.append(ml4.astype(BF))
        m_hi.append((1.0 - ml4).astype(BF))

    return dict(
        node_of_row=node_of_row,
        row_of_node=row_of_node,
        idx=idx_cores, oh=oh_cores, oht=oht_cores,
        idxlo=lo_cores, idxhi=hi_cores, mlo=m_lo, mhi=m_hi,
        BA=BA, BB=BB,
    )


def _blkdiag(a):
    out = np.zeros((HC, HEADS), np.float32)
    a = np.asarray(a, np.float32)
    for h in range(HEADS):
        out[h * CH : (h + 1) * CH, h] = a[h]
    return out


def build_inputs(cfg: Cfg, layout, x, W1, a_src1, a_dst1, b1, W2, a_src2,
                 a_dst2, b2, fc_w, fc_b):
    node_of_row = layout["node_of_row"]
    xs = np.zeros((cfg.rows, DIN), np.float32)
    valid = node_of_row >= 0
    xs[valid] = np.asarray(x, np.float32)[node_of_row[valid]]
    xbf = np.ascontiguousarray(xs.T).astype(BF)            # [128, rows]

    W1 = np.asarray(W1, np.float32)
    W2 = np.asarray(W2, np.float32)
    w1aug = np.concatenate(
        [W1, W1 @ _blkdiag(a_src1), W1 @ _blkdiag(a_dst1)], axis=1).astype(BF)
    w2full = np.concatenate(
        [W2, W2 @ _blkdiag(a_src2), W2 @ _blkdiag(a_dst2)], axis=1).astype(BF)
    w2aug = np.ascontiguousarray(w2full.reshape(2, 128, USED))

    def _bexp(b):
        # [128, (half, wi, d)] pre-expanded bias
        bc = np.asarray(b, np.float32).reshape(2, 128).T    # [128, half]
        return np.ascontiguousarray(
            np.broadcast_to(bc[:, :, None, None], (128, 2, 2, 64))
            .reshape(128, 256))

    b1c = _bexp(b1)
    b2c = _bexp(b2)
    fcw = np.ascontiguousarray(
        np.asarray(fc_w, np.float32).reshape(2, 128, 1).astype(BF))
    fcb = np.full((128, 1), np.float32(np.asarray(fc_b).reshape(-1)[0]))

    sel = np.zeros((2, 4, 128), np.float32)
    for half in range(2):
        for h in range(2):
            sel[half, 2 * half + h, h * CH : (h + 1) * CH] = 1.0
    sel = np.ascontiguousarray(sel)

    eye = np.zeros((2, 128, 64), np.float32)
    eye[0, np.arange(64), np.arange(64)] = 1.0
    eye[1, 64 + np.arange(64), np.arange(64)] = 1.0
    eye = eye.astype(BF)

    base = dict(xbf=xbf, w1aug=w1aug, w2aug=w2aug, b1c=b1c, b2c=b2c,
                fcw=fcw, fcb=fcb, sel=sel, eye=eye)
    in_maps = []
    assert cfg.cch == 1
    for c in range(cfg.nc):
        m = dict(base)
        m["idx"] = layout["idx"][c]
        m["oh"] = layout["oh"][c]
        m["oht"] = layout["oht"][c]
        # own-node inputs x (feature-major) for the direct adW1 projection
        m["xown"] = np.ascontiguousarray(
            xs[c * cfg.pcn : (c + 1) * cfg.pcn].T).astype(BF)
        in_maps.append(m)
    return in_maps


def build_program(cfg: Cfg, shared_out: bool = True):
    nc_b = bacc.Bacc(None, num_devices=cfg.nc)
    NC, ND, W, GCH = cfg.nc, cfg.ndst, cfg.wpc, cfg.gch
    nA, nB = cfg.nA, cfg.nB
    BA, BB = int(np.sum(nA)), int(np.sum(nB))
    NBLK = BA + BB
    ROWS = cfg.rows
    PCN = cfg.pcn
    NSUB = PCN // 128                                   # own-gather sub count
    offA = np.concatenate([[0], np.cumsum(nA)]).astype(int)
    offB = np.concatenate([[0], np.cumsum(nB)]).astype(int)

    xbfT = nc_b.dram_tensor("xbf", [DIN, ROWS], BF16, kind="ExternalInput")
    w1augT = nc_b.dram_tensor("w1aug", [DIN, USED], BF16, kind="ExternalInput")
    w2augT = nc_b.dram_tensor("w2aug", [2, 128, USED], BF16, kind="ExternalInput")
    b1cT = nc_b.dram_tensor("b1c", [128, 256], F32, kind="ExternalInput")
    b2cT = nc_b.dram_tensor("b2c", [128, 256], F32, kind="ExternalInput")
    fcwT = nc_b.dram_tensor("fcw", [2, 128, 1], BF16, kind="ExternalInput")
    fcbT = nc_b.dram_tensor("fcb", [128, 1], F32, kind="ExternalInput")
    selT = nc_b.dram_tensor("sel", [2, 4, 128], F32, kind="ExternalInput")
    idxT = nc_b.dram_tensor("idx", [128, NBLK * 8], I16, kind="ExternalInput")
    ohT = nc_b.dram_tensor("oh", [128, NBLK * ND], F8, kind="ExternalInput")
    ohtT = nc_b.dram_tensor("oht", [ND, NBLK * 128], F8, kind="ExternalInput")
    xownT = nc_b.dram_tensor("xown", [DIN, PCN], BF16, kind="ExternalInput")
    eyeT = nc_b.dram_tensor("eye", [2, 128, 64], BF16, kind="ExternalInput")
    yT = nc_b.dram_tensor("y", [128, W // 2], F32, kind="ExternalOutput")

    HPCg = PCN // cfg.cch
    HALFg = ROWS // cfg.cch
    table1 = nc_b.dram_tensor("table1", [ROWS, EW], BF16)
    tab2own_t = [
        nc_b.dram_tensor(f"tab2own{k}", [HPCg, EW], BF16)
        for k in range(cfg.cch)
    ]
    table2_t = [
        nc_b.dram_tensor(
            f"table2_{k}", [HALFg, EW], BF16,
            addr_space="Shared" if shared_out else "Local")
        for k in range(cfg.cch)
    ]

    # chunk plan over the A-stream then B-stream of blocks
    chunks = []
    for base, nb in ((0, BA), (BA, BB)):
        b = 0
        while b < nb:
            wdt = min(GCH, nb - b)
            chunks.append((base + b, wdt, base == BA))
            b += wdt
    chunk_of_blk = {}
    for ci, (b0, cw, _) in enumerate(chunks):
        for k in range(cw):
            chunk_of_blk[b0 + k] = (ci, k)

    win_of_blk = np.zeros(NBLK, int)
    for w in range(W):
        win_of_blk[offA[w] : offA[w + 1]] = w
        win_of_blk[BA + offB[w] : BA + offB[w + 1]] = w

    CPW = W // cfg.cch

    import contextlib

    with tile.TileContext(nc_b) as tc:
        ctx = [
            tc.tile_pool(name="cst", bufs=1),
            tc.tile_pool(name="dx", bufs=POOLS["dx"]),
            tc.tile_pool(name="dh", bufs=POOLS["dh"]),
            tc.tile_pool(name="ixp", bufs=3),
            tc.tile_pool(name="gtA", bufs=TUNE["gt"]),
            tc.tile_pool(name="gtB", bufs=TUNE["gt"]),
            tc.tile_pool(name="er", bufs=TUNE["er"]),
            tc.tile_pool(name="e4A", bufs=TUNE["e4"]),
            tc.tile_pool(name="e4B", bufs=TUNE["e4"]),
            tc.tile_pool(name="og", bufs=2),
            tc.tile_pool(name="ohA", bufs=TUNE["oh"]),
            tc.tile_pool(name="ohB", bufs=TUNE["oh"]),
            tc.tile_pool(name="otA", bufs=TUNE["ot"]),
            tc.tile_pool(name="otB", bufs=TUNE["ot"]),
            tc.tile_pool(name="sm", bufs=TUNE["sm"]),
            tc.tile_pool(name="fin", bufs=TUNE["fin"]),
            tc.tile_pool(name="sp", bufs=1),
            tc.tile_pool(name="pa", bufs=POOLS["pa"], space="PSUM"),
            tc.tile_pool(name="pg", bufs=POOLS["pg"], space="PSUM"),
            tc.tile_pool(name="pp", bufs=POOLS["pp"], space="PSUM"),
        ]
        with contextlib.ExitStack() as st:
            (cst, dx, dh, ixp, gtA, gtB, er, e4A, e4B, og,
             ohA, ohB, otA, otB, sm, fin, sp,
             pa, pg, pp) = [st.enter_context(m) for m in ctx]

            # ---- constants ----
            w1s = cst.tile([128, USED], BF16)
            nc_b.sync.dma_start(out=w1s[:], in_=w1augT[:, :])
            w2s = cst.tile([128, 2, USED], BF16)
            nc_b.sync.dma_start(out=w2s[:, 0, :], in_=w2augT[0, :, :])
            nc_b.sync.dma_start(out=w2s[:, 1, :], in_=w2augT[1, :, :])
            b1s = cst.tile([128, 256], F32)
            nc_b.sync.dma_start(out=b1s[:], in_=b1cT[:, :])
            b2s = cst.tile([128, 256], F32)
            nc_b.sync.dma_start(out=b2s[:], in_=b2cT[:, :])
            fcws = cst.tile([128, 2, 1], BF16)
            nc_b.sync.dma_start(out=fcws[:, 0, :], in_=fcwT[0, :, :])
            nc_b.sync.dma_start(out=fcws[:, 1, :], in_=fcwT[1, :, :])
            fcbs = cst.tile([128, 1], F32)
            nc_b.sync.dma_start(out=fcbs[:], in_=fcbT[:, :])
            sels = cst.tile([4, 2, 128], F32)
            nc_b.sync.dma_start(out=sels[:, 0, :], in_=selT[0, :, :])
            nc_b.sync.dma_start(out=sels[:, 1, :], in_=selT[1, :, :])
            ixs = cst.tile([128, NBLK * 8], I16)
            nc_b.sync.dma_start(out=ixs[:], in_=idxT[:, :])
            xos = cst.tile([128, PCN], BF16)
            nc_b.sync.dma_start(out=xos[:], in_=xownT[:, :])
            eyes = cst.tile([128, 2, 64], BF16)
            nc_b.sync.dma_start(out=eyes[:, 0, :], in_=eyeT[0, :, :])
            nc_b.sync.dma_start(out=eyes[:, 1, :], in_=eyeT[1, :, :])
            adW1 = cst.tile([ND, W, HEADS], BF16)
            adW2 = cst.tile([ND, W, HEADS], BF16)
            zAll = cst.tile([128, W // 2], F32)
            negones = cst.tile([128, 1], F32)
            nc_b.vector.memset(negones[:], -1.0)
            adraw2 = cst.tile([128, W // 2, HEADS], BF16)

            # ---- dense phase (replicated): table1 rows = [x @ W1aug] ----
            NT8 = ROWS // 1024
            for t8 in range(NT8):
                xin = dx.tile([128, 8, 128], BF16, tag="xin")
                nc_b.sync.dma_start(
                    out=xin[:], in_=xbfT[:, t8 * 1024 : (t8 + 1) * 1024])
                hb = dh.tile([128, 8, USED], BF16, tag="hb")
                for i in range(8):
                    pht = pp.tile([128, USED], F32, space="PSUM", tag="pp")
                    nc_b.tensor.matmul(
                        out=pht[:], lhsT=xin[:, i, :], rhs=w1s[:],
                        start=True, stop=True)
                    if i % 2 == 0:
                        nc_b.vector.tensor_copy(out=hb[:, i, :], in_=pht[:])
                    else:
                        nc_b.scalar.copy(out=hb[:, i, :], in_=pht[:])
                nc_b.sync.dma_start(
                    out=bass.AP(
                        tensor=table1[:, :].tensor,
                        offset=t8 * 1024 * EW,
                        ap=[[EW, 128], [EW * 128, 8], [1, USED]],
                    ),
                    in_=bass.AP(
                        tensor=hb[:].tensor, offset=hb[:].offset,
                        ap=[hb[:].ap[0], [USED, 8], [1, USED]],
                    ),
                )

            # ---- adW1 directly from x_own @ (W1 a_dst1) ----
            # ad1[node] is a projection of the INPUT x, so it needs neither
            # table1 nor any own-row gather: 49 tiny PE matmuls against the
            # ad columns of w1s.  This unblocks the Pool queue -- layer-1
            # edge gathers now start as soon as table1's A-range is written.
            adraw = cst.tile([128, NSUB, HEADS], BF16)
            adO = pa.tile([128, 196], F32, space="PSUM", tag="ad")
            for p in range(NSUB):
                nc_b.tensor.matmul(
                    out=adO[:, p * HEADS : (p + 1) * HEADS],
                    lhsT=xos[:, p * 128 : (p + 1) * 128],
                    rhs=w1s[:, AS_OFF + HEADS : USED],
                    start=True, stop=True)
            nc_b.vector.tensor_copy(
                out=bass.AP(
                    tensor=adraw[:].tensor, offset=adraw[:].offset,
                    ap=[adraw[:].ap[0], [1, NSUB * HEADS]],
                ),
                in_=adO[:, 0 : NSUB * HEADS])
            for par in range(2):
                adT = pa.tile([128, 196], F32, space="PSUM", tag="ad")
                adPP = adT[0:ND, 0 : NSUB * HEADS]
                nc_b.tensor.matmul(
                    out=adPP, lhsT=eyes[:, par, :], rhs=adraw[:],
                    start=True, stop=True)
                nc_b.vector.tensor_copy(
                    out=bass.AP(
                        tensor=adW1[:].tensor,
                        offset=adW1[:].offset + par * HEADS,
                        ap=[adW1[:].ap[0], [2 * HEADS, NSUB], [1, HEADS]],
                    ),
                    in_=adPP)

            def cc_chunk(k):
                nc_b.gpsimd.collective_compute(
                    "AllGather",
                    OP.bypass,
                    replica_groups=[list(range(NC))],
                    ins=[tab2own_t[k][:, :].opt()],
                    outs=[table2_t[k][:, :].opt()],
                )

            def mk_state(layer):
                st = dict(
                    emitted=[False] * len(chunks),
                    preloaded=[None] * len(chunks),
                    preadp=[None] * len(chunks),
                    ctiles=[None] * len(chunks),
                )

                def emit_loads(ci):
                    # SP-only part: prefetchable many chunks ahead
                    if st["preloaded"][ci] is not None:
                        return st["preloaded"][ci]
                    b0, cw, is_b = chunks[ci]
                    oh_p = ohB if is_b else ohA
                    ot_p = otB if is_b else otA
                    oh_t = oh_p.tile([128, GCH * ND], F8, tag="oh")
                    nc_b.sync.dma_start(
                        out=oh_t[:, 0 : cw * ND],
                        in_=ohT[:, b0 * ND : (b0 + cw) * ND])
                    oht_t = ot_p.tile([ND, GCH * 128], F8, tag="oht")
                    nc_b.sync.dma_start(
                        out=oht_t[:, 0 : cw * 128],
                        in_=ohtT[:, b0 * 128 : (b0 + cw) * 128])
                    st["preloaded"][ci] = (oh_t, oht_t)
                    return st["preloaded"][ci]

                def emit_adp(ci):
                    # PE part, done near emit_chunk time (bounded PSUM use)
                    if st["preadp"][ci] is not None:
                        return st["preadp"][ci]
                    b0, cw, is_b = chunks[ci]
                    adW = adW1 if layer == 1 else adW2
                    oh_t, oht_t = emit_loads(ci)
                    adT = pa.tile([128, 196], F32, space="PSUM", tag="ad")
                    adP = adT[:, 0 : GCH * HEADS]
                    for k in range(cw):
                        wv = int(win_of_blk[b0 + k])
                        nc_b.tensor.matmul(
                            out=adP[:, k * HEADS : (k + 1) * HEADS],
                            lhsT=oht_t[:, k * 128 : (k + 1) * 128],
                            rhs=adW[:, wv, :],
                            start=True, stop=True)
                    st["preadp"][ci] = (oh_t, adP)
                    return st["preadp"][ci]

                st["emit_loads"] = emit_loads
                st["emit_adp"] = emit_adp
                return st

            # chunk index ranges per side (for load lookahead)
            nA_chunks = len([c for c in chunks if not c[2]])

            def edge_layer(layer, st, pre_cc=None):
                emitted = st["emitted"]
                ctiles = st["ctiles"]
                emit_loads = st["emit_loads"]
                emit_adp = st["emit_adp"]

                def emit_chunk(ci):
                    b0, cw, is_b = chunks[ci]
                    gt_p = gtB if is_b else gtA
                    e4_p = e4B if is_b else e4A
                    # prefetch loads for upcoming chunks of this side
                    lo, hi = ((nA_chunks, len(chunks)) if is_b
                              else (0, nA_chunks))
                    for cj in range(ci, min(ci + LOOKAHEAD, hi)):
                        emit_loads(cj)
                    oh_t, adP = emit_adp(ci)
                    for cj in range(ci + 1, min(ci + ADLA + 1, hi)):
                        emit_adp(cj)
                    g = gt_p.tile([128, GCH, EW], BF16, tag="g")
                    if layer == 1:
                        tab = table1
                    elif cfg.cch == 1:
                        tab = table2_t[0]
                    else:
                        assert ASPLIT == HALFg
                        tab = table2_t[1 if is_b else 0]
                    if layer == 1 or cfg.cch == 1:
                        in_ap = (tab[ASPLIT:ROWS, :] if is_b
                                 else tab[0:ASPLIT, :])
                    else:
                        in_ap = tab[:, :]
                    for q0 in range(0, cw, GQ):
                        qw = min(GQ, cw - q0)
                        nc_b.gpsimd.dma_gather(
                            out_ap=bass.AP(
                                tensor=g[:].tensor,
                                offset=g[:].offset + q0 * EW,
                                ap=[g[:].ap[0], [EW, qw], [1, EW]],
                            ),
                            in_ap=in_ap,
                            idxs_ap=ixs[:, (b0 + q0) * 8 : (b0 + q0 + qw) * 8],
                            num_idxs=128 * qw,
                            num_idxs_reg=128 * qw,
                            elem_size=EW,
                        )
                    e0 = sm.tile([128, GCH * HEADS], BF16, tag="e0")
                    nc_b.vector.tensor_tensor(
                        out=e0[:, 0 : cw * HEADS],
                        in0=bass.AP(
                            tensor=g[:].tensor, offset=g[:].offset + AS_OFF,
                            ap=[g[:].ap[0], [EW, cw], [1, HEADS]],
                        ),
                        in1=adP[:, 0 : cw * HEADS],
                        op=OP.add)
                    e2 = sm.tile([128, GCH * HEADS], BF16, tag="e2")
                    nc_b.vector.scalar_tensor_tensor(
                        out=e2[:, 0 : cw * HEADS], in0=e0[:, 0 : cw * HEADS],
                        scalar=NEG, in1=e0[:, 0 : cw * HEADS],
                        op0=OP.mult, op1=OP.max)
                    # exp broadcast split: ACT fills cols [0:ES), DVE
                    # replicates the rest with a 4x-mode copy
                    ert = er.tile([128, GCH, HEADS, CH], BF16, tag="er")
                    nc_b.scalar.activation(
                        out=bass.AP(
                            tensor=ert[:].tensor, offset=ert[:].offset,
                            ap=[ert[:].ap[0], [CH, cw * HEADS], [1, ES]],
                        ),
                        in_=bass.AP(
                            tensor=e2[:].tensor, offset=e2[:].offset,
                            ap=[e2[:].ap[0], [1, cw * HEADS], [0, ES]],
                        ),
                        func=ACT.Exp)
                    if ES < CH:
                        nc_b.vector.tensor_copy(
                            out=bass.AP(
                                tensor=ert[:].tensor,
                                offset=ert[:].offset + ES,
                                ap=[ert[:].ap[0], [CH, cw * HEADS],
                                    [1, CH - ES]],
                            ),
                            in_=bass.AP(
                                tensor=ert[:].tensor, offset=ert[:].offset,
                                ap=[ert[:].ap[0], [CH, cw * HEADS],
                                    [1, CH - ES]],
                            ))
                    # scale h in place inside the gather tile (saves SBUF)
                    gpt = g
                    nc_b.vector.tensor_tensor(
                        out=bass.AP(
                            tensor=g[:].tensor, offset=g[:].offset,
                            ap=[g[:].ap[0], [EW, cw], [1, HC]],
                        ),
                        in0=bass.AP(
                            tensor=g[:].tensor, offset=g[:].offset,
                            ap=[g[:].ap[0], [EW, cw], [1, HC]],
                        ),
                        in1=ert[:, 0:cw, :, :],
                        op=OP.mult)
                    exp4 = e4_p.tile([128, GCH, HEADS], BF16, tag="e4")
                    nc_b.vector.tensor_copy(
                        out=exp4[:, 0:cw, :],
                        in_=bass.AP(
                            tensor=ert[:].tensor, offset=ert[:].offset,
                            ap=[ert[:].ap[0], [HEADS * CH, cw], [CH, HEADS]],
                        ))
                    emitted[ci] = True
                    ctiles[ci] = (gpt, exp4, oh_t)

                # windows processed in PAIRS: one PSUM tile + one finalize
                # chain per two windows (layout [128, wi, {seg0,seg1,den}, ND])
                for w in range(0, W, 2):
                    # PSUM layout [128, {seg_half0, seg_half1, den}, wi, ND]
                    segF = pg.tile([128, 3, 2, ND], F32, space="PSUM",
                                   tag="seg")
                    first_mm = True
                    lasts = []
                    for wi in range(2):
                        bl = list(range(offA[w + wi], offA[w + wi + 1])) + \
                             list(range(BA + offB[w + wi],
                                        BA + offB[w + wi + 1]))
                        lasts.append(bl[-1])
                    for wi in range(2):
                        bl = list(range(offA[w + wi], offA[w + wi + 1])) + \
                             list(range(BA + offB[w + wi],
                                        BA + offB[w + wi + 1]))
                        for b in bl:
                            ci, k = chunk_of_blk[b]
                            if not emitted[ci]:
                                emit_chunk(ci)
                            gpt, exp4, oh_t = ctiles[ci]
                            for half in range(2):
                                nc_b.tensor.matmul(
                                    out=segF[:, half, wi, :],
                                    lhsT=bass.AP(
                                        tensor=gpt[:].tensor,
                                        offset=(gpt[:].offset + k * EW
                                                + half * 128),
                                        ap=[gpt[:].ap[0], [1, 128]],
                                    ),
                                    rhs=oh_t[:, k * ND : (k + 1) * ND],
                                    start=first_mm, stop=False)
                                first_mm = False
                            nc_b.tensor.matmul(
                                out=segF[0:4, 2, wi, :],
                                lhsT=exp4[:, k, :],
                                rhs=oh_t[:, k * ND : (k + 1) * ND],
                                start=False,
                                stop=(wi == 1 and b == lasts[1]))

                    # ---- finalize window pair (w, w+1) ----
                    den_s = fin.tile([4, 2, ND], F32, tag="dens")
                    nc_b.vector.reciprocal(
                        out=den_s[:], in_=segF[0:4, 2, :, :])
                    rdT = pa.tile([128, 2, 2, ND], F32, space="PSUM",
                                  tag="ad")
                    for half in range(2):
                        nc_b.tensor.matmul(
                            out=rdT[:, half, :, :],
                            lhsT=sels[:, half, :],
                            rhs=den_s[:],
                            start=True, stop=True)
                    # flat [128, (wi, half, d)] layout for finalize tensors
                    rdenS = fin.tile([128, 2 * 2 * ND], F32, tag="rdenS")
                    nc_b.scalar.copy(
                        out=rdenS[:],
                        in_=bass.AP(
                            tensor=rdT[:].tensor, offset=rdT[:].offset,
                            ap=[rdT[:].ap[0], [1, 2 * 2 * ND]],
                        ))
                    nrm = fin.tile([128, 2 * 2 * ND], F32, tag="nrm")
                    nc_b.vector.tensor_tensor(
                        out=nrm[:],
                        in0=bass.AP(
                            tensor=segF[:].tensor, offset=segF[:].offset,
                            ap=[segF[:].ap[0], [1, 2 * 2 * ND]],
                        ),
                        in1=rdenS[:], op=OP.mult)
                    bsel = b1s if layer == 1 else b2s
                    xb = fin.tile([128, 2 * 2 * ND], F32, tag="xb")
                    nc_b.vector.tensor_tensor(
                        out=xb[:], in0=nrm[:], in1=bsel[:], op=OP.add)
                    em = fin.tile([128, 2 * 2 * ND], F32, tag="em")
                    nc_b.scalar.activation(out=em[:], in_=xb[:], func=ACT.Exp)
                    rl = fin.tile([128, 2 * 2 * ND], F32, tag="rl")
                    nc_b.vector.scalar_tensor_tensor(
                        out=rl[:], in0=xb[:], scalar=0.0,
                        in1=bass.AP(
                            tensor=negones[:].tensor,
                            offset=negones[:].offset,
                            ap=[negones[:].ap[0], [0, 2 * 2 * ND]],
                        ),
                        op0=OP.max, op1=OP.add)
                    eluT = fin.tile([128, 2, 2, ND], BF16, tag="eluT")
                    nc_b.vector.scalar_tensor_tensor(
                        out=bass.AP(
                            tensor=eluT[:].tensor, offset=eluT[:].offset,
                            ap=[eluT[:].ap[0], [1, 2 * 2 * ND]],
                        ),
                        in0=em[:], scalar=1.0, in1=rl[:],
                        op0=OP.min, op1=OP.add)
                    ppt = pp.tile([128, USED], F32, space="PSUM", tag="pp")
                    if layer == 1:
                        h2P = ppt[:, :]
                        for half in range(2):
                            nc_b.tensor.matmul(
                                out=h2P,
                                lhsT=eluT[:, half, :, :],
                                rhs=w2s[:, half, :],
                                start=(half == 0), stop=(half == 1))
                        h2b = fin.tile([128, EW], BF16, tag="h2b")
                        nc_b.scalar.copy(out=h2b[:, 0:USED], in_=h2P)
                        # stash this pair's alpha_dst rows ([wi,d] partitions)
                        nc_b.sync.dma_start(
                            out=adraw2[:, w // 2, :],
                            in_=h2b[:, AD_OFF:USED])
                        nc_b.sync.dma_start(
                            out=tab2own_t[0][w * ND : (w + 2) * ND, :],
                            in_=h2b[:])
                        if w + 2 == W:
                            for par in range(2):
                                adT2 = pa.tile([128, 196], F32,
                                               space="PSUM", tag="ad")
                                adPP2 = adT2[0:ND, 0 : (W // 2) * HEADS]
                                nc_b.tensor.matmul(
                                    out=adPP2, lhsT=eyes[:, par, :],
                                    rhs=adraw2[:],
                                    start=True, stop=True)
                                nc_b.vector.tensor_copy(
                                    out=bass.AP(
                                        tensor=adW2[:].tensor,
                                        offset=(adW2[:].offset
                                                + par * HEADS),
                                        ap=[adW2[:].ap[0],
                                            [2 * HEADS, W // 2],
                                            [1, HEADS]],
                                    ),
                                    in_=adPP2)
                            if pre_cc is not None:
                                pre_cc()
                            cc_chunk(0)
                    else:
                        zP = ppt[:, 0:1]
                        for half in range(2):
                            nc_b.tensor.matmul(
                                out=zP,
                                lhsT=eluT[:, half, :, :],
                                rhs=fcws[:, half, :],
                                start=(half == 0), stop=(half == 1))
                        nc_b.vector.tensor_copy(
                            out=zAll[:, w // 2 : w // 2 + 1], in_=zP)

            marks = {}
            nc_b._phase_marks = marks
            st1 = mk_state(1)
            st2 = mk_state(2)
            firstB = len([c for c in chunks if not c[2]])

            def prefetch2():
                # stream layer-2 one-hot loads under the collective
                for ci in range(min(CCPRE, firstB)):
                    st2["emit_loads"](ci)
                for ci in range(firstB, min(firstB + CCPRE, len(chunks))):
                    st2["emit_loads"](ci)

            marks["setup_end"] = len(nc_b.inst_map)
            edge_layer(1, st1, pre_cc=prefetch2)
            marks["e1_end"] = len(nc_b.inst_map)
            edge_layer(2, st2)
            marks["e2_end"] = len(nc_b.inst_map)

            ysig = cst.tile([128, W // 2], F32)
            nc_b.scalar.activation(
                out=ysig[:], in_=zAll[:], func=ACT.Sigmoid,
                bias=fcbs[:, 0:1], scale=1.0)
            nc_b.sync.dma_start(out=yT[:, :], in_=ysig[:])

    nc_b.finalize()
    return nc_b


def assemble_output(cfg: Cfg, layout, results):
    node_of_row = layout["node_of_row"]
    yfull = np.zeros((cfg.n_real, 1), np.float32)
    CPW = cfg.wpc // cfg.cch
    HPC = cfg.pcn // cfg.cch
    HALF = cfg.rows // cfg.cch
    for c in range(cfg.nc):
        yc = np.asarray(results[c]["y"])               # [128, 49]
        q, p = np.meshgrid(np.arange(128), np.arange(cfg.wpc // 2),
                           indexing="ij")
        w = 2 * p + q // cfg.ndst
        d = q % cfg.ndst
        hh = w // CPW
        rows = hh * HALF + c * HPC + (w - hh * CPW) * cfg.ndst + d
        nodes = node_of_row[rows.reshape(-1)]
        ok = nodes >= 0
        yfull[nodes[ok], 0] = yc.reshape(-1)[ok]
    return yfull


def _absorb_device_wedge():
    """Run a trivial 8-core kernel; a crashed prior session can leave the
    NeuronCores in NRT_EXEC_UNIT_UNRECOVERABLE state, which a fresh trivial
    execution clears."""
    try:
        from concourse.bass_utils import run_bass_kernel_spmd

        nc_t = bacc.Bacc(None, num_devices=8)
        a = nc_t.dram_tensor("a", [128, 128], F32, kind="ExternalInput")
        o = nc_t.dram_tensor("o", [128, 128], F32, kind="ExternalOutput")
        with tile.TileContext(nc_t) as tc:
            with tc.tile_pool(name="sb", bufs=1) as sb:
                t = sb.tile([128, 128], F32)
                nc_t.sync.dma_start(out=t[:], in_=a[:, :])
                nc_t.sync.dma_start(out=o[:, :], in_=t[:])
        nc_t.finalize()
        run_bass_kernel_spmd(
            nc_t, [{"a": np.zeros((128, 128), np.float32)}] * 8,
            core_ids=list(range(8)),
        )
    except Exception:
        pass


def kernel(**inputs):
    from concourse.bass_utils import run_bass_kernel_spmd

    cfg = Cfg()
    layout = build_layout(inputs["edge_index"], cfg)
    in_maps = build_inputs(
        cfg, layout,
        inputs["x"], inputs["W1"], inputs["a_src1"], inputs["a_dst1"],
        inputs["b1"], inputs["W2"], inputs["a_src2"], inputs["a_dst2"],
        inputs["b2"], inputs["fc_w"], inputs["fc_b"],
    )
    nc_b = build_program(cfg, shared_out=True)
    last_err = None
    for attempt in range(3):
        try:
            res = run_bass_kernel_spmd(
                nc_b, in_maps, core_ids=list(range(cfg.nc)))
            return assemble_output(cfg, layout, res.results)
        except Exception as e:  # wedged device from a prior crashed session
            last_err = e
            _absorb_device_wedge()
    raise last_err


if __name__ == "__main__":
    pass



# revision 106
# speedup vs baseline: 1.0162x; 1.0018x over previous
"""Two-layer GAT (PyG GATConv-style) on 8 Trainium2 NeuronCores via Bass/Tile.

Edges-on-partitions design (v3, pair-window pipeline):
  - Nodes are degree-stratified into 98 strata of 512; each stratum contributes
    64 nodes to every core (snake order), giving core-major table rows
    row = core*6272 + window*64 + slot.  Window = 64 destination nodes.
  - A bf16 feature table holds rows [h(256) | alpha_src(4) | alpha_dst(4) | pad]
    with 768 B stride.  Layer-1 table is computed replicated (dense bf16
    matmuls); layer-2 table rows are produced per window pair by a fused dense
    matmul and AllGathered across the 8 cores (the AllGather hard-blocks the
    Pool queue in the cost model, so a single cch=1 collective is optimal).
  - Each core processes its ~106k incoming edges as 128-edge blocks (sorted by
    window, split into A/B streams at ASPLIT=25088 so gather indices fit
    int16).  SWDGE dma_gather fetches h[src] rows for gch=7 blocks per
    instruction; the steady state is gather-stream bound (back-to-back).
  - Attention: alpha_dst per edge via one-hot^T matmul (one-hots stored in
    fp8e4 -- PE matmuls take mixed bf16/fp8 operands); e = lrelu(as+ad) on
    DVE; exp broadcast to [4,64] on Activation; G' = G * exp multiplied IN
    PLACE in the gather tile (DVE 2x); segment sum + softmax denominator via
    PE matmuls accumulated per WINDOW PAIR in one PSUM tile
    [128, {seg0,seg1,den}, wi, 64].  No segment-max needed.
  - Pair finalize (flat [128,(half,wi,d)] layout, >=3-dim-AP verifier-safe):
    1/den via reciprocal + sel-matmul broadcast + one ACT copy; bias add from
    a pre-expanded [128,256] bias tile; ELU as min(exp(xb),1) + (max(xb,0)-1);
    h2/fc matmuls cover both windows in one 128-partition lhsT.  alpha_dst
    for layer 2 is stashed per pair via SBUF->SBUF DMA of the h2b ad columns
    and assembled with eyes-matmuls at end of layer 1; for layer 1 it comes
    from own-row gathers that fetch only the 256 B row tail (elem_step=EW).
  - Load pipeline: one-hot loads prefetched LOOKAHEAD chunks ahead, adP
    matmuls ADLA chunks ahead, CCPRE layer-2 load chunks streamed under the
    collective.  One Sigmoid at the very end ([128, 49] pair-major output).
"""

import sys

sys.path.insert(0, "/opt/trn_rl_repo")

from dataclasses import dataclass, field

import numpy as np
import ml_dtypes

BF = ml_dtypes.bfloat16
F8NP = ml_dtypes.float8_e4m3fn

import concourse.bass as bass
import concourse.bacc as bacc
import concourse.tile as tile
from concourse import mybir

F32 = mybir.dt.float32
BF16 = mybir.dt.bfloat16
F8 = mybir.dt.float8e4
I16 = mybir.dt.int16
OP = mybir.AluOpType
ACT = mybir.ActivationFunctionType

HEADS = 4
CH = 64
HC = 256
DIN = 128
NEG = 0.2
EW = 384                 # table row width in bf16 elems (768 B)
USED = 264               # used columns: h(256) + as(4) + ad(4)
AS_OFF = 256
AD_OFF = 260
ASPLIT = 25088           # first B-range table row (= chunk boundary for cch=2)
HISTART = 50176 - 32768  # = 17408, start row of the hi own-gather range
TUNE = dict(gt=8, gp=2, e4=6, oh=6, ot=6, er=5, fin=3, sm=8)
GQ = 16                  # blocks per dma_gather instruction
LOOKAHEAD = 3            # one-hot load prefetch depth (chunks, per side)
CCPRE = 12               # chunks of L2 loads streamed under the collective
ADLA = 2                 # adP matmul lookahead (chunks)
ES = 64                  # exp-broadcast columns computed on ACT (rest: DVE)
POOLS = dict(dx=4, dh=4, pa=3, pg=2, pp=3)


@dataclass
class Cfg:
    n_real: int = 50000
    nc: int = 8
    ndst: int = 64               # dst nodes per window
    wpc: int = 98                # windows per core
    gch: int = 7                 # blocks per chunk tile
    cch: int = 1                 # collective chunks for table2
    nA: list = field(default_factory=list)   # per-window A-block counts
    nB: list = field(default_factory=list)   # per-window B-block counts
    # kept for test.py compatibility (prints sumK)
    ka: list = field(default_factory=list)
    kb: list = field(default_factory=list)

    @property
    def pcn(self):
        return self.wpc * self.ndst          # nodes per core (6272)

    @property
    def rows(self):
        return self.nc * self.pcn            # table rows (50176)


def _pack_idx(blk):
    """blk: [nblk, 128] int16 -> wrapped-16 layout [128, nblk*8]."""
    nblk = blk.shape[0]
    pk = blk.reshape(nblk, 8, 16).transpose(2, 0, 1).reshape(16, nblk * 8)
    return np.ascontiguousarray(np.tile(pk, (8, 1)).astype(np.int16))


def build_layout(edge_index, cfg: Cfg):
    n = cfg.n_real
    NC, ND, W = cfg.nc, cfg.ndst, cfg.wpc
    src = np.asarray(edge_index[0], dtype=np.int64)
    dst = np.asarray(edge_index[1], dtype=np.int64)
    src = np.concatenate([src, np.arange(n, dtype=np.int64)])
    dst = np.concatenate([dst, np.arange(n, dtype=np.int64)])
    deg = np.bincount(dst, minlength=n)

    order = np.argsort(-deg, kind="stable")          # degree-descending
    node_of_row = np.full(cfg.rows, -1, np.int64)
    row_of_node = np.full(n, -1, np.int64)
    j = np.arange(512)
    r8 = j // 8
    c8 = j % 8
    core_j = np.where(r8 % 2 == 0, c8, 7 - c8)
    slot_j = r8
    CPW = W // cfg.cch                      # windows per collective chunk
    HPC = cfg.pcn // cfg.cch                # rows per core per chunk (3136)
    HALF = cfg.rows // cfg.cch              # rows per chunk (25088)
    for s in range(W):
        nodes = order[s * 512 : (s + 1) * 512]
        hh = s // CPW
        rows = (hh * HALF + core_j[: len(nodes)] * HPC
                + (s - hh * CPW) * ND + slot_j[: len(nodes)])
        node_of_row[rows] = nodes
        row_of_node[nodes] = rows

    drow = row_of_node[dst]
    hh_e = drow // HALF
    rem = drow % HALF
    core_e = rem // HPC
    loc2 = rem % HPC
    w_e = hh_e * CPW + loc2 // ND
    dloc = loc2 % ND
    srow = row_of_node[src]
    sideB = srow >= ASPLIT

    cntA = np.zeros((NC, W), np.int64)
    cntB = np.zeros((NC, W), np.int64)
    np.add.at(cntA, (core_e[~sideB], w_e[~sideB]), 1)
    np.add.at(cntB, (core_e[sideB], w_e[sideB]), 1)
    nA = np.maximum(1, np.ceil(cntA.max(axis=0) / 128).astype(np.int64))
    nB = np.maximum(1, np.ceil(cntB.max(axis=0) / 128).astype(np.int64))
    cfg.nA = nA.tolist()
    cfg.nB = nB.tolist()
    cfg.ka = nA.tolist()
    cfg.kb = nB.tolist()
    BA, BB = int(nA.sum()), int(nB.sum())
    offA = np.concatenate([[0], np.cumsum(nA)]).astype(int)
    offB = np.concatenate([[0], np.cumsum(nB)]).astype(int)

    eorder = np.lexsort((w_e, sideB, core_e))
    srow_s = srow[eorder]
    dloc_s = dloc[eorder]
    core_s = core_e[eorder]
    sideB_s = sideB[eorder]
    w_s = w_e[eorder]
    cstarts = np.searchsorted(core_s, np.arange(NC + 1))

    idx_cores, oh_cores, oht_cores = [], [], []
    lo_cores, hi_cores, m_lo, m_hi = [], [], [], []
    for c in range(NC):
        lo_, hi_ = cstarts[c], cstarts[c + 1]
        sr_c = srow_s[lo_:hi_]
        dl_c = dloc_s[lo_:hi_]
        sd_c = sideB_s[lo_:hi_]
        ww_c = w_s[lo_:hi_]
        idx_blk = np.zeros((BA + BB, 128), np.int16)
        dl_blk = np.full((BA + BB, 128), -1, np.int64)
        bstart = np.searchsorted(sd_c, 1)
        for sideflag, nW, off, base, elo, ehi in (
            (False, nA, offA, 0, 0, bstart),
            (True, nB, offB, BA, bstart, len(sr_c)),
        ):
            sr = sr_c[elo:ehi] - (ASPLIT if sideflag else 0)
            dl = dl_c[elo:ehi]
            ww = ww_c[elo:ehi]
            starts = np.searchsorted(ww, np.arange(W + 1))
            for w in range(W):
                s0, s1 = starts[w], starts[w + 1]
                cnt = s1 - s0
                b0 = base + off[w]
                fa = idx_blk[b0 : b0 + nW[w]].reshape(-1)
                fa[:cnt] = sr[s0:s1]
                fd = dl_blk[b0 : b0 + nW[w]].reshape(-1)
                fd[:cnt] = dl[s0:s1]
        idx_cores.append(_pack_idx(idx_blk))
        ohb = np.zeros((BA + BB, 128, ND), np.uint8)
        bb, pp = np.nonzero(dl_blk >= 0)
        ohb[bb, pp, dl_blk[bb, pp]] = 1
        oh_cores.append(np.ascontiguousarray(
            ohb.transpose(1, 0, 2).reshape(128, -1).astype(F8NP)))
        oht_cores.append(np.ascontiguousarray(
            ohb.transpose(2, 0, 1).reshape(ND, -1).astype(F8NP)))

        # own-row gather indices (for layer-1 alpha_dst): lo/hi + masks.
        # own position j = w*ND + d; row depends on the chunk-major layout.
        jj = np.arange(cfg.pcn)
        wn = jj // ND
        hh = wn // CPW
        own = hh * HALF + c * HPC + (wn - hh * CPW) * ND + (jj % ND)
        is_lo = own < ASPLIT
        lo_idx = np.where(is_lo, own, 0).astype(np.int16)
        hi_idx = np.where(~is_lo, own - HISTART, 0).astype(np.int16)
        lo_cores.append(_pack_idx(lo_idx.reshape(-1, 128)))
        hi_cores.append(_pack_idx(hi_idx.reshape(-1, 128)))
        # mask per position, laid out [partition, sub, head]
        ml = is_lo.astype(np.float32)
        ml4 = np.repeat(ml[:, None], HEADS, 1).reshape(-1, 128, HEADS)
        ml4 = ml4.transpose(1, 0, 2).reshape(128, -1)
        m_lo.append(ml4.astype(BF))
        m_hi.append((1.0 - ml4).astype(BF))

    return dict(
        node_of_row=node_of_row,
        row_of_node=row_of_node,
        idx=idx_cores, oh=oh_cores, oht=oht_cores,
        idxlo=lo_cores, idxhi=hi_cores, mlo=m_lo, mhi=m_hi,
        BA=BA, BB=BB,
    )


def _blkdiag(a):
    out = np.zeros((HC, HEADS), np.float32)
    a = np.asarray(a, np.float32)
    for h in range(HEADS):
        out[h * CH : (h + 1) * CH, h] = a[h]
    return out


def build_inputs(cfg: Cfg, layout, x, W1, a_src1, a_dst1, b1, W2, a_src2,
                 a_dst2, b2, fc_w, fc_b):
    node_of_row = layout["node_of_row"]
    xs = np.zeros((cfg.rows, DIN), np.float32)
    valid = node_of_row >= 0
    xs[valid] = np.asarray(x, np.float32)[node_of_row[valid]]
    xbf = np.ascontiguousarray(xs.T).astype(BF)            # [128, rows]

    W1 = np.asarray(W1, np.float32)
    W2 = np.asarray(W2, np.float32)
    w1aug = np.concatenate(
        [W1, W1 @ _blkdiag(a_src1), W1 @ _blkdiag(a_dst1)], axis=1).astype(BF)
    w2full = np.concatenate(
        [W2, W2 @ _blkdiag(a_src2), W2 @ _blkdiag(a_dst2)], axis=1).astype(BF)
    w2aug = np.ascontiguousarray(w2full.reshape(2, 128, USED))

    def _bexp(b):
        # [128, (half, wi, d)] pre-expanded bias
        bc = np.asarray(b, np.float32).reshape(2, 128).T    # [128, half]
        return np.ascontiguousarray(
            np.broadcast_to(bc[:, :, None, None], (128, 2, 2, 64))
            .reshape(128, 256))

    b1c = _bexp(b1)
    b2c = _bexp(b2)
    fcw = np.ascontiguousarray(
        np.asarray(fc_w, np.float32).reshape(2, 128, 1).astype(BF))
    fcb = np.full((128, 1), np.float32(np.asarray(fc_b).reshape(-1)[0]))

    sel = np.zeros((2, 4, 128), np.float32)
    for half in range(2):
        for h in range(2):
            sel[half, 2 * half + h, h * CH : (h + 1) * CH] = 1.0
    sel = np.ascontiguousarray(sel)

    eye = np.zeros((2, 128, 64), np.float32)
    eye[0, np.arange(64), np.arange(64)] = 1.0
    eye[1, 64 + np.arange(64), np.arange(64)] = 1.0
    eye = eye.astype(BF)

    base = dict(xbf=xbf, w1aug=w1aug, w2aug=w2aug, b1c=b1c, b2c=b2c,
                fcw=fcw, fcb=fcb, sel=sel, eye=eye)
    in_maps = []
    assert cfg.cch == 1
    for c in range(cfg.nc):
        m = dict(base)
        m["idx"] = layout["idx"][c]
        m["oh"] = layout["oh"][c]
        m["oht"] = layout["oht"][c]
        # own-node inputs x (feature-major) for the direct adW1 projection
        m["xown"] = np.ascontiguousarray(
            xs[c * cfg.pcn : (c + 1) * cfg.pcn].T).astype(BF)
        in_maps.append(m)
    return in_maps


def build_program(cfg: Cfg, shared_out: bool = True):
    nc_b = bacc.Bacc(None, num_devices=cfg.nc)
    NC, ND, W, GCH = cfg.nc, cfg.ndst, cfg.wpc, cfg.gch
    nA, nB = cfg.nA, cfg.nB
    BA, BB = int(np.sum(nA)), int(np.sum(nB))
    NBLK = BA + BB
    ROWS = cfg.rows
    PCN = cfg.pcn
    NSUB = PCN // 128                                   # own-gather sub count
    offA = np.concatenate([[0], np.cumsum(nA)]).astype(int)
    offB = np.concatenate([[0], np.cumsum(nB)]).astype(int)

    xbfT = nc_b.dram_tensor("xbf", [DIN, ROWS], BF16, kind="ExternalInput")
    w1augT = nc_b.dram_tensor("w1aug", [DIN, USED], BF16, kind="ExternalInput")
    w2augT = nc_b.dram_tensor("w2aug", [2, 128, USED], BF16, kind="ExternalInput")
    b1cT = nc_b.dram_tensor("b1c", [128, 256], F32, kind="ExternalInput")
    b2cT = nc_b.dram_tensor("b2c", [128, 256], F32, kind="ExternalInput")
    fcwT = nc_b.dram_tensor("fcw", [2, 128, 1], BF16, kind="ExternalInput")
    fcbT = nc_b.dram_tensor("fcb", [128, 1], F32, kind="ExternalInput")
    selT = nc_b.dram_tensor("sel", [2, 4, 128], F32, kind="ExternalInput")
    idxT = nc_b.dram_tensor("idx", [128, NBLK * 8], I16, kind="ExternalInput")
    ohT = nc_b.dram_tensor("oh", [128, NBLK * ND], F8, kind="ExternalInput")
    ohtT = nc_b.dram_tensor("oht", [ND, NBLK * 128], F8, kind="ExternalInput")
    xownT = nc_b.dram_tensor("xown", [DIN, PCN], BF16, kind="ExternalInput")
    eyeT = nc_b.dram_tensor("eye", [2, 128, 64], BF16, kind="ExternalInput")
    yT = nc_b.dram_tensor("y", [128, W // 2], F32, kind="ExternalOutput")

    HPCg = PCN // cfg.cch
    HALFg = ROWS // cfg.cch
    table1 = nc_b.dram_tensor("table1", [ROWS, EW], BF16)
    tab2own_t = [
        nc_b.dram_tensor(f"tab2own{k}", [HPCg, EW], BF16)
        for k in range(cfg.cch)
    ]
    table2_t = [
        nc_b.dram_tensor(
            f"table2_{k}", [HALFg, EW], BF16,
            addr_space="Shared" if shared_out else "Local")
        for k in range(cfg.cch)
    ]

    # chunk plan over the A-stream then B-stream of blocks
    chunks = []
    for base, nb in ((0, BA), (BA, BB)):
        b = 0
        while b < nb:
            wdt = min(GCH, nb - b)
            chunks.append((base + b, wdt, base == BA))
            b += wdt
    chunk_of_blk = {}
    for ci, (b0, cw, _) in enumerate(chunks):
        for k in range(cw):
            chunk_of_blk[b0 + k] = (ci, k)

    win_of_blk = np.zeros(NBLK, int)
    for w in range(W):
        win_of_blk[offA[w] : offA[w + 1]] = w
        win_of_blk[BA + offB[w] : BA + offB[w + 1]] = w

    CPW = W // cfg.cch

    import contextlib

    with tile.TileContext(nc_b) as tc:
        ctx = [
            tc.tile_pool(name="cst", bufs=1),
            tc.tile_pool(name="dx", bufs=POOLS["dx"]),
            tc.tile_pool(name="dh", bufs=POOLS["dh"]),
            tc.tile_pool(name="ixp", bufs=3),
            tc.tile_pool(name="gtA", bufs=TUNE["gt"]),
            tc.tile_pool(name="gtB", bufs=TUNE["gt"]),
            tc.tile_pool(name="er", bufs=TUNE["er"]),
            tc.tile_pool(name="e4A", bufs=TUNE["e4"]),
            tc.tile_pool(name="e4B", bufs=TUNE["e4"]),
            tc.tile_pool(name="og", bufs=2),
            tc.tile_pool(name="ohA", bufs=TUNE["oh"]),
            tc.tile_pool(name="ohB", bufs=TUNE["oh"]),
            tc.tile_pool(name="otA", bufs=TUNE["ot"]),
            tc.tile_pool(name="otB", bufs=TUNE["ot"]),
            tc.tile_pool(name="sm", bufs=TUNE["sm"]),
            tc.tile_pool(name="fin", bufs=TUNE["fin"]),
            tc.tile_pool(name="sp", bufs=1),
            tc.tile_pool(name="pa", bufs=POOLS["pa"], space="PSUM"),
            tc.tile_pool(name="pg", bufs=POOLS["pg"], space="PSUM"),
            tc.tile_pool(name="pp", bufs=POOLS["pp"], space="PSUM"),
        ]
        with contextlib.ExitStack() as st:
            (cst, dx, dh, ixp, gtA, gtB, er, e4A, e4B, og,
             ohA, ohB, otA, otB, sm, fin, sp,
             pa, pg, pp) = [st.enter_context(m) for m in ctx]

            # ---- constants ----
            w1s = cst.tile([128, USED], BF16)
            nc_b.sync.dma_start(out=w1s[:], in_=w1augT[:, :])
            w2s = cst.tile([128, 2, USED], BF16)
            nc_b.sync.dma_start(out=w2s[:, 0, :], in_=w2augT[0, :, :])
            nc_b.sync.dma_start(out=w2s[:, 1, :], in_=w2augT[1, :, :])
            b1s = cst.tile([128, 256], F32)
            nc_b.sync.dma_start(out=b1s[:], in_=b1cT[:, :])
            b2s = cst.tile([128, 256], F32)
            nc_b.sync.dma_start(out=b2s[:], in_=b2cT[:, :])
            fcws = cst.tile([128, 2, 1], BF16)
            nc_b.sync.dma_start(out=fcws[:, 0, :], in_=fcwT[0, :, :])
            nc_b.sync.dma_start(out=fcws[:, 1, :], in_=fcwT[1, :, :])
            fcbs = cst.tile([128, 1], F32)
            nc_b.sync.dma_start(out=fcbs[:], in_=fcbT[:, :])
            sels = cst.tile([4, 2, 128], F32)
            nc_b.sync.dma_start(out=sels[:, 0, :], in_=selT[0, :, :])
            nc_b.sync.dma_start(out=sels[:, 1, :], in_=selT[1, :, :])
            ixs = cst.tile([128, NBLK * 8], I16)
            nc_b.sync.dma_start(out=ixs[:], in_=idxT[:, :])
            xos = cst.tile([128, PCN], BF16)
            nc_b.sync.dma_start(out=xos[:], in_=xownT[:, :])
            eyes = cst.tile([128, 2, 64], BF16)
            nc_b.sync.dma_start(out=eyes[:, 0, :], in_=eyeT[0, :, :])
            nc_b.sync.dma_start(out=eyes[:, 1, :], in_=eyeT[1, :, :])
            adW1 = cst.tile([ND, W, HEADS], BF16)
            adW2 = cst.tile([ND, W, HEADS], BF16)
            zAll = cst.tile([128, W // 2], F32)
            negones = cst.tile([128, 1], F32)
            nc_b.vector.memset(negones[:], -1.0)
            adraw2 = cst.tile([128, W // 2, HEADS], BF16)

            # ---- dense phase (replicated): table1 rows = [x @ W1aug] ----
            NT8 = ROWS // 1024
            for t8 in range(NT8):
                xin = dx.tile([128, 8, 128], BF16, tag="xin")
                nc_b.sync.dma_start(
                    out=xin[:], in_=xbfT[:, t8 * 1024 : (t8 + 1) * 1024])
                hb = dh.tile([128, 8, USED], BF16, tag="hb")
                for i in range(8):
                    pht = pp.tile([128, USED], F32, space="PSUM", tag="pp")
                    nc_b.tensor.matmul(
                        out=pht[:], lhsT=xin[:, i, :], rhs=w1s[:],
                        start=True, stop=True)
                    if i % 2 == 0:
                        nc_b.vector.tensor_copy(out=hb[:, i, :], in_=pht[:])
                    else:
                        nc_b.scalar.copy(out=hb[:, i, :], in_=pht[:])
                nc_b.sync.dma_start(
                    out=bass.AP(
                        tensor=table1[:, :].tensor,
                        offset=t8 * 1024 * EW,
                        ap=[[EW, 128], [EW * 128, 8], [1, USED]],
                    ),
                    in_=bass.AP(
                        tensor=hb[:].tensor, offset=hb[:].offset,
                        ap=[hb[:].ap[0], [USED, 8], [1, USED]],
                    ),
                )

            # ---- adW1 directly from x_own @ (W1 a_dst1) ----
            # ad1[node] is a projection of the INPUT x, so it needs neither
            # table1 nor any own-row gather: 49 tiny PE matmuls against the
            # ad columns of w1s.  This unblocks the Pool queue -- layer-1
            # edge gathers now start as soon as table1's A-range is written.
            adraw = cst.tile([128, NSUB, HEADS], BF16)
            adO = pa.tile([128, 196], F32, space="PSUM", tag="ad")
            for p in range(NSUB):
                nc_b.tensor.matmul(
                    out=adO[:, p * HEADS : (p + 1) * HEADS],
                    lhsT=xos[:, p * 128 : (p + 1) * 128],
                    rhs=w1s[:, AS_OFF + HEADS : USED],
                    start=True, stop=True)
            nc_b.vector.tensor_copy(
                out=bass.AP(
                    tensor=adraw[:].tensor, offset=adraw[:].offset,
                    ap=[adraw[:].ap[0], [1, NSUB * HEADS]],
                ),
                in_=adO[:, 0 : NSUB * HEADS])
            for par in range(2):
                adT = pa.tile([128, 196], F32, space="PSUM", tag="ad")
                adPP = adT[0:ND, 0 : NSUB * HEADS]
                nc_b.tensor.matmul(
                    out=adPP, lhsT=eyes[:, par, :], rhs=adraw[:],
                    start=True, stop=True)
                nc_b.vector.tensor_copy(
                    out=bass.AP(
                        tensor=adW1[:].tensor,
                        offset=adW1[:].offset + par * HEADS,
                        ap=[adW1[:].ap[0], [2 * HEADS, NSUB], [1, HEADS]],
                    ),
                    in_=adPP)

            def cc_chunk(k):
                nc_b.gpsimd.collective_compute(
                    "AllGather",
                    OP.bypass,
                    replica_groups=[list(range(NC))],
                    ins=[tab2own_t[k][:, :].opt()],
                    outs=[table2_t[k][:, :].opt()],
                )

            def mk_state(layer):
                st = dict(
                    emitted=[False] * len(chunks),
                    preloaded=[None] * len(chunks),
                    preadp=[None] * len(chunks),
                    ctiles=[None] * len(chunks),
                )

                def emit_loads(ci):
                    # SP-only part: prefetchable many chunks ahead
                    if st["preloaded"][ci] is not None:
                        return st["preloaded"][ci]
                    b0, cw, is_b = chunks[ci]
                    oh_p = ohB if is_b else ohA
                    ot_p = otB if is_b else otA
                    oh_t = oh_p.tile([128, GCH * ND], F8, tag="oh")
                    nc_b.sync.dma_start(
                        out=oh_t[:, 0 : cw * ND],
                        in_=ohT[:, b0 * ND : (b0 + cw) * ND])
                    oht_t = ot_p.tile([ND, GCH * 128], F8, tag="oht")
                    nc_b.sync.dma_start(
                        out=oht_t[:, 0 : cw * 128],
                        in_=ohtT[:, b0 * 128 : (b0 + cw) * 128])
                    st["preloaded"][ci] = (oh_t, oht_t)
                    return st["preloaded"][ci]

                def emit_adp(ci):
                    # PE part, done near emit_chunk time (bounded PSUM use)
                    if st["preadp"][ci] is not None:
                        return st["preadp"][ci]
                    b0, cw, is_b = chunks[ci]
                    adW = adW1 if layer == 1 else adW2
                    oh_t, oht_t = emit_loads(ci)
                    adT = pa.tile([128, 196], F32, space="PSUM", tag="ad")
                    adP = adT[:, 0 : GCH * HEADS]
                    for k in range(cw):
                        wv = int(win_of_blk[b0 + k])
                        nc_b.tensor.matmul(
                            out=adP[:, k * HEADS : (k + 1) * HEADS],
                            lhsT=oht_t[:, k * 128 : (k + 1) * 128],
                            rhs=adW[:, wv, :],
                            start=True, stop=True)
                    st["preadp"][ci] = (oh_t, adP)
                    return st["preadp"][ci]

                st["emit_loads"] = emit_loads
                st["emit_adp"] = emit_adp
                return st

            # chunk index ranges per side (for load lookahead)
            nA_chunks = len([c for c in chunks if not c[2]])

            def edge_layer(layer, st, pre_cc=None):
                emitted = st["emitted"]
                ctiles = st["ctiles"]
                emit_loads = st["emit_loads"]
                emit_adp = st["emit_adp"]

                def emit_chunk(ci):
                    b0, cw, is_b = chunks[ci]
                    gt_p = gtB if is_b else gtA
                    e4_p = e4B if is_b else e4A
                    # prefetch loads for upcoming chunks of this side
                    lo, hi = ((nA_chunks, len(chunks)) if is_b
                              else (0, nA_chunks))
                    for cj in range(ci, min(ci + LOOKAHEAD, hi)):
                        emit_loads(cj)
                    oh_t, adP = emit_adp(ci)
                    for cj in range(ci + 1, min(ci + ADLA + 1, hi)):
                        emit_adp(cj)
                    g = gt_p.tile([128, GCH, EW], BF16, tag="g")
                    if layer == 1:
                        tab = table1
                    elif cfg.cch == 1:
                        tab = table2_t[0]
                    else:
                        assert ASPLIT == HALFg
                        tab = table2_t[1 if is_b else 0]
                    if layer == 1 or cfg.cch == 1:
                        in_ap = (tab[ASPLIT:ROWS, :] if is_b
                                 else tab[0:ASPLIT, :])
                    else:
                        in_ap = tab[:, :]
                    for q0 in range(0, cw, GQ):
                        qw = min(GQ, cw - q0)
                        nc_b.gpsimd.dma_gather(
                            out_ap=bass.AP(
                                tensor=g[:].tensor,
                                offset=g[:].offset + q0 * EW,
                                ap=[g[:].ap[0], [EW, qw], [1, EW]],
                            ),
                            in_ap=in_ap,
                            idxs_ap=ixs[:, (b0 + q0) * 8 : (b0 + q0 + qw) * 8],
                            num_idxs=128 * qw,
                            num_idxs_reg=128 * qw,
                            elem_size=EW,
                        )
                    e0 = sm.tile([128, GCH * HEADS], BF16, tag="e0")
                    nc_b.vector.tensor_tensor(
                        out=e0[:, 0 : cw * HEADS],
                        in0=bass.AP(
                            tensor=g[:].tensor, offset=g[:].offset + AS_OFF,
                            ap=[g[:].ap[0], [EW, cw], [1, HEADS]],
                        ),
                        in1=adP[:, 0 : cw * HEADS],
                        op=OP.add)
                    e2 = sm.tile([128, GCH * HEADS], BF16, tag="e2")
                    nc_b.vector.scalar_tensor_tensor(
                        out=e2[:, 0 : cw * HEADS], in0=e0[:, 0 : cw * HEADS],
                        scalar=NEG, in1=e0[:, 0 : cw * HEADS],
                        op0=OP.mult, op1=OP.max)
                    # exp broadcast split: ACT fills cols [0:ES), DVE
                    # replicates the rest with a 4x-mode copy
                    ert = er.tile([128, GCH, HEADS, CH], BF16, tag="er")
                    nc_b.scalar.activation(
                        out=bass.AP(
                            tensor=ert[:].tensor, offset=ert[:].offset,
                            ap=[ert[:].ap[0], [CH, cw * HEADS], [1, ES]],
                        ),
                        in_=bass.AP(
                            tensor=e2[:].tensor, offset=e2[:].offset,
                            ap=[e2[:].ap[0], [1, cw * HEADS], [0, ES]],
                        ),
                        func=ACT.Exp)
                    if ES < CH:
                        nc_b.vector.tensor_copy(
                            out=bass.AP(
                                tensor=ert[:].tensor,
                                offset=ert[:].offset + ES,
                                ap=[ert[:].ap[0], [CH, cw * HEADS],
                                    [1, CH - ES]],
                            ),
                            in_=bass.AP(
                                tensor=ert[:].tensor, offset=ert[:].offset,
                                ap=[ert[:].ap[0], [CH, cw * HEADS],
                                    [1, CH - ES]],
                            ))
                    # scale h in place inside the gather tile (saves SBUF)
                    gpt = g
                    nc_b.vector.tensor_tensor(
                        out=bass.AP(
                            tensor=g[:].tensor, offset=g[:].offset,
                            ap=[g[:].ap[0], [EW, cw], [1, HC]],
                        ),
                        in0=bass.AP(
                            tensor=g[:].tensor, offset=g[:].offset,
                            ap=[g[:].ap[0], [EW, cw], [1, HC]],
                        ),
                        in1=ert[:, 0:cw, :, :],
                        op=OP.mult)
                    exp4 = e4_p.tile([128, GCH, HEADS], BF16, tag="e4")
                    nc_b.vector.tensor_copy(
                        out=exp4[:, 0:cw, :],
                        in_=bass.AP(
                            tensor=ert[:].tensor, offset=ert[:].offset,
                            ap=[ert[:].ap[0], [HEADS * CH, cw], [CH, HEADS]],
                        ))
                    emitted[ci] = True
                    ctiles[ci] = (gpt, exp4, oh_t)

                # windows processed in PAIRS: one PSUM tile + one finalize
                # chain per two windows (layout [128, wi, {seg0,seg1,den}, ND])
                for w in range(0, W, 2):
                    # PSUM layout [128, {seg_half0, seg_half1, den}, wi, ND]
                    segF = pg.tile([128, 3, 2, ND], F32, space="PSUM",
                                   tag="seg")
                    first_mm = True
                    lasts = []
                    for wi in range(2):
                        bl = list(range(offA[w + wi], offA[w + wi + 1])) + \
                             list(range(BA + offB[w + wi],
                                        BA + offB[w + wi + 1]))
                        lasts.append(bl[-1])
                    for wi in range(2):
                        bl = list(range(offA[w + wi], offA[w + wi + 1])) + \
                             list(range(BA + offB[w + wi],
                                        BA + offB[w + wi + 1]))
                        for b in bl:
                            ci, k = chunk_of_blk[b]
                            if not emitted[ci]:
                                emit_chunk(ci)
                            gpt, exp4, oh_t = ctiles[ci]
                            for half in range(2):
                                nc_b.tensor.matmul(
                                    out=segF[:, half, wi, :],
                                    lhsT=bass.AP(
                                        tensor=gpt[:].tensor,
                                        offset=(gpt[:].offset + k * EW
                                                + half * 128),
                                        ap=[gpt[:].ap[0], [1, 128]],
                                    ),
                                    rhs=oh_t[:, k * ND : (k + 1) * ND],
                                    start=first_mm, stop=False)
                                first_mm = False
                            nc_b.tensor.matmul(
                                out=segF[0:4, 2, wi, :],
                                lhsT=exp4[:, k, :],
                                rhs=oh_t[:, k * ND : (k + 1) * ND],
                                start=False,
                                stop=(wi == 1 and b == lasts[1]))

                    # ---- finalize window pair (w, w+1) ----
                    den_s = fin.tile([4, 2, ND], F32, tag="dens")
                    nc_b.vector.reciprocal(
                        out=den_s[:], in_=segF[0:4, 2, :, :])
                    rdT = pa.tile([128, 2, 2, ND], F32, space="PSUM",
                                  tag="ad")
                    for half in range(2):
                        nc_b.tensor.matmul(
                            out=rdT[:, half, :, :],
                            lhsT=sels[:, half, :],
                            rhs=den_s[:],
                            start=True, stop=True)
                    # flat [128, (wi, half, d)] layout for finalize tensors
                    rdenS = fin.tile([128, 2 * 2 * ND], F32, tag="rdenS")
                    nc_b.scalar.copy(
                        out=rdenS[:],
                        in_=bass.AP(
                            tensor=rdT[:].tensor, offset=rdT[:].offset,
                            ap=[rdT[:].ap[0], [1, 2 * 2 * ND]],
                        ))
                    nrm = fin.tile([128, 2 * 2 * ND], F32, tag="nrm")
                    nc_b.vector.tensor_tensor(
                        out=nrm[:],
                        in0=bass.AP(
                            tensor=segF[:].tensor, offset=segF[:].offset,
                            ap=[segF[:].ap[0], [1, 2 * 2 * ND]],
                        ),
                        in1=rdenS[:], op=OP.mult)
                    bsel = b1s if layer == 1 else b2s
                    xb = fin.tile([128, 2 * 2 * ND], F32, tag="xb")
                    nc_b.vector.tensor_tensor(
                        out=xb[:], in0=nrm[:], in1=bsel[:], op=OP.add)
                    em = fin.tile([128, 2 * 2 * ND], F32, tag="em")
                    nc_b.scalar.activation(out=em[:], in_=xb[:], func=ACT.Exp)
                    rl = fin.tile([128, 2 * 2 * ND], F32, tag="rl")
                    nc_b.vector.scalar_tensor_tensor(
                        out=rl[:], in0=xb[:], scalar=0.0,
                        in1=bass.AP(
                            tensor=negones[:].tensor,
                            offset=negones[:].offset,
                            ap=[negones[:].ap[0], [0, 2 * 2 * ND]],
                        ),
                        op0=OP.max, op1=OP.add)
                    eluT = fin.tile([128, 2, 2, ND], BF16, tag="eluT")
                    nc_b.vector.scalar_tensor_tensor(
                        out=bass.AP(
                            tensor=eluT[:].tensor, offset=eluT[:].offset,
                            ap=[eluT[:].ap[0], [1, 2 * 2 * ND]],
                        ),
                        in0=em[:], scalar=1.0, in1=rl[:],
                        op0=OP.min, op1=OP.add)
                    ppt = pp.tile([128, USED], F32, space="PSUM", tag="pp")
                    if layer == 1:
                        h2P = ppt[:, :]
                        for half in range(2):
                            nc_b.tensor.matmul(
                                out=h2P,
                                lhsT=eluT[:, half, :, :],
                                rhs=w2s[:, half, :],
                                start=(half == 0), stop=(half == 1))
                        h2b = fin.tile([128, EW], BF16, tag="h2b")
                        nc_b.scalar.copy(out=h2b[:, 0:USED], in_=h2P)
                        # stash this pair's alpha_dst rows ([wi,d] partitions)
                        nc_b.sync.dma_start(
                            out=adraw2[:, w // 2, :],
                            in_=h2b[:, AD_OFF:USED])
                        nc_b.sync.dma_start(
                            out=tab2own_t[0][w * ND : (w + 2) * ND, :],
                            in_=h2b[:])
                        if w + 2 == W:
                            for par in range(2):
                                adT2 = pa.tile([128, 196], F32,
                                               space="PSUM", tag="ad")
                                adPP2 = adT2[0:ND, 0 : (W // 2) * HEADS]
                                nc_b.tensor.matmul(
                                    out=adPP2, lhsT=eyes[:, par, :],
                                    rhs=adraw2[:],
                                    start=True, stop=True)
                                nc_b.vector.tensor_copy(
                                    out=bass.AP(
                                        tensor=adW2[:].tensor,
                                        offset=(adW2[:].offset
                                                + par * HEADS),
                                        ap=[adW2[:].ap[0],
                                            [2 * HEADS, W // 2],
                                            [1, HEADS]],
                                    ),
                                    in_=adPP2)
                            if pre_cc is not None:
                                pre_cc()
                            cc_chunk(0)
                    else:
                        zP = ppt[:, 0:1]
                        for half in range(2):
                            nc_b.tensor.matmul(
                                out=zP,
                                lhsT=eluT[:, half, :, :],
                                rhs=fcws[:, half, :],
                                start=(half == 0), stop=(half == 1))
                        nc_b.vector.tensor_copy(
                            out=zAll[:, w // 2 : w // 2 + 1], in_=zP)

            marks = {}
            nc_b._phase_marks = marks
            st1 = mk_state(1)
            st2 = mk_state(2)
            firstB = len([c for c in chunks if not c[2]])

            def prefetch2():
                # stream layer-2 one-hot loads under the collective
                for ci in range(min(CCPRE, firstB)):
                    st2["emit_loads"](ci)
                for ci in range(firstB, min(firstB + CCPRE, len(chunks))):
                    st2["emit_loads"](ci)

            marks["setup_end"] = len(nc_b.inst_map)
            edge_layer(1, st1, pre_cc=prefetch2)
            marks["e1_end"] = len(nc_b.inst_map)
            edge_layer(2, st2)
            marks["e2_end"] = len(nc_b.inst_map)

            ysig = cst.tile([128, W // 2], F32)
            nc_b.scalar.activation(
                out=ysig[:], in_=zAll[:], func=ACT.Sigmoid,
                bias=fcbs[:, 0:1], scale=1.0)
            nc_b.sync.dma_start(out=yT[:, :], in_=ysig[:])

    nc_b.finalize()
    return nc_b


def assemble_output(cfg: Cfg, layout, results):
    node_of_row = layout["node_of_row"]
    yfull = np.zeros((cfg.n_real, 1), np.float32)
    CPW = cfg.wpc // cfg.cch
    HPC = cfg.pcn // cfg.cch
    HALF = cfg.rows // cfg.cch
    for c in range(cfg.nc):
        yc = np.asarray(results[c]["y"])               # [128, 49]
        q, p = np.meshgrid(np.arange(128), np.arange(cfg.wpc // 2),
                           indexing="ij")
        w = 2 * p + q // cfg.ndst
        d = q % cfg.ndst
        hh = w // CPW
        rows = hh * HALF + c * HPC + (w - hh * CPW) * cfg.ndst + d
        nodes = node_of_row[rows.reshape(-1)]
        ok = nodes >= 0
        yfull[nodes[ok], 0] = yc.reshape(-1)[ok]
    return yfull


def _absorb_device_wedge():
    """Run a trivial 8-core kernel; a crashed prior session can leave the
    NeuronCores in NRT_EXEC_UNIT_UNRECOVERABLE state, which a fresh trivial
    execution clears."""
    try:
        from concourse.bass_utils import run_bass_kernel_spmd

        nc_t = bacc.Bacc(None, num_devices=8)
        a = nc_t.dram_tensor("a", [128, 128], F32, kind="ExternalInput")
        o = nc_t.dram_tensor("o", [128, 128], F32, kind="ExternalOutput")
        with tile.TileContext(nc_t) as tc:
            with tc.tile_pool(name="sb", bufs=1) as sb:
                t = sb.tile([128, 128], F32)
                nc_t.sync.dma_start(out=t[:], in_=a[:, :])
                nc_t.sync.dma_start(out=o[:, :], in_=t[:])
        nc_t.finalize()
        run_bass_kernel_spmd(
            nc_t, [{"a": np.zeros((128, 128), np.float32)}] * 8,
            core_ids=list(range(8)),
        )
    except Exception:
        pass


def kernel(**inputs):
    from concourse.bass_utils import run_bass_kernel_spmd

    cfg = Cfg()
    layout = build_layout(inputs["edge_index"], cfg)
    in_maps = build_inputs(
        cfg, layout,
        inputs["x"], inputs["W1"], inputs["a_src1"], inputs["a_dst1"],
        inputs["b1"], inputs["W2"], inputs["a_src2"], inputs["a_dst2"],
        inputs["b2"], inputs["fc_w"], inputs["fc_b"],
    )
    nc_b = build_program(cfg, shared_out=True)
    last_err = None
    for attempt in range(3):
        try:
            res = run_bass_kernel_spmd(
                nc_b, in_maps, core_ids=list(range(cfg.nc)))
            return assemble_output(cfg, layout, res.results)
        except Exception as e:  # wedged device from a prior crashed session
            last_err = e
            _absorb_device_wedge()
    raise last_err


if __name__ == "__main__":
    pass

